# revision 26
# baseline (speedup 1.0000x reference)
"""ContactLoss Trainium2 kernel v3.2 (8 NeuronCores, batch data-parallel,
spatially-pruned KNN).

Big pass (minho): hand verts kd-sorted into sub-chunks of 32; valid obj
verts kd-sorted into 32-col blocks. Host computes exact lower bounds
(point-to-block-bbox) and upper bounds (dist to obj reps) and keeps, per
sub-chunk, only the obj blocks that can contain a nearest neighbour.
Needed blocks are host-gathered into dense 512-col banks. On device,
16-way PE tiling (32x32 diagonal tiles, tile_position=(32c,32c))
computes 4 sub-chunks (one per 32-lane class) concurrently. Schedule is
rank-pure: rank r = the r-th-largest sub-chunk of each class (sorted by
bank count); nbk[slot][r] = max bank count over classes and cores, so a
single SPMD program serves all 8 cores. Wave = 4 consecutive banks; one
[22,32]x[22,512] matmul per (bank, class); consume reduces the wave to
per-bank minima columns of MP; one tiny contiguous reduce per rank
collapses its banks into MH, then masked sums produce the three loss
numerators. Small pass (minoh for interior obj): same machinery, roles
swapped. Classes pad short ranks by repeating their own blocks
(min-safe).

d2 precision: xx+yy-2xy with bf16 split products (6 pairs). The lhsT
side carries only the leading bf16 term of its squared-norm (x0 big /
y0 small); the residual (x1+x2 or y1+y2) is constant per output lane,
so it is added after the min on-device (keeps the f16 drain centred
near zero). COORD_SCALE keeps f16 drains in range.
"""

import sys
from contextlib import ExitStack

import numpy as np

sys.path.insert(0, "/opt/trn_rl_repo")

import concourse.mybir as mybir  # noqa: E402
import concourse.tile as tile  # noqa: E402
from concourse import bacc  # noqa: E402
from concourse.bass_utils import run_bass_kernel_spmd  # noqa: E402

B, NH, NO = 32, 778, 8192
NCORES = 8
BPC = B // NCORES
SC = 32  # sub-chunk lanes
NSC = (NH + SC - 1) // SC  # 25 hand sub-chunks
FB = 32  # fine block cols for the need test / gather granularity
BK = 512  # PSUM bank columns
HB = 256  # half-bank columns (scheduling granularity)
BPH = HB // FB  # fine blocks per half-bank
KD = 22
CS = np.float32(16.0)
BIG = np.float32(49152.0)
PAD = np.float32(8192.0)
NHP = ((NH + FB - 1) // FB) * FB  # 800

F32 = mybir.dt.float32
F16 = mybir.dt.float16
BF16 = mybir.dt.bfloat16
MIN = mybir.AluOpType.min
MULT = mybir.AluOpType.mult
ADD = mybir.AluOpType.add
AX = mybir.AxisListType.X
AF = mybir.ActivationFunctionType

_nc_cache = {}


# ---------------------------------------------------------------- geometry
def _kd_order(pts, leaf):
    """Recursive median split on widest axis -> contiguous leaves."""
    out = []

    def rec(ids):
        if len(ids) <= leaf:
            out.append(ids)
            return
        p = pts[ids]
        ax = int(np.argmax(p.max(0) - p.min(0)))
        k = len(ids) // 2
        part = np.argpartition(p[:, ax], k)
        rec(ids[part[:k]])
        rec(ids[part[k:]])

    sys.setrecursionlimit(100000)
    rec(np.arange(len(pts)))
    return np.concatenate(out)


def _box_dist2(q, blo, bhi):
    d = np.maximum(np.maximum(blo[None, :, :] - q[:, None, :],
                              q[:, None, :] - bhi[None, :, :]), 0.0)
    return (d * d).sum(-1)


def _needed_blocks(lanes_pts, sorted_pts, reps):
    """Per sub-chunk-of-32 of lanes_pts: indices of needed FB-blocks."""
    n = len(sorted_pts)
    starts = np.arange(0, n, FB)
    blo = np.minimum.reduceat(sorted_pts, starts)
    bhi = np.maximum.reduceat(sorted_pts, starts)
    d2 = ((lanes_pts[:, None, :] - reps[None, :, :]) ** 2).sum(-1)
    u2 = d2.min(1)
    lb2 = _box_dist2(lanes_pts, blo, bhi)
    need = lb2 <= u2[:, None] * (1.0 + 1e-9) + 1e-12
    nsub = (len(lanes_pts) + SC - 1) // SC
    return [np.nonzero(need[s * SC:(s + 1) * SC].any(0))[0]
            for s in range(nsub)]


# ---------------------------------------------------------------- device
def _build(plan):
    nc = bacc.Bacc("TRN2", target_bir_lowering=False, debug=False,
                   num_devices=NCORES)
    TOTBb, TOTBs = plan["totb_big"], plan["totb_small"]
    Rb, Rs = plan["r_big"], plan["r_small"]

    rb_d = nc.declare_dram_parameter("rb", [4, KD, TOTBb * HB], BF16,
                                     isOutput=False)
    wb_d = nc.declare_dram_parameter("wb", [4, KD, Rb * SC], BF16,
                                     isOutput=False)
    rs_d = nc.declare_dram_parameter("rs", [4, KD, TOTBs * HB], BF16,
                                     isOutput=False)
    ws_d = nc.declare_dram_parameter("ws", [4, KD, Rs * SC], BF16,
                                     isOutput=False)
    me_d = nc.declare_dram_parameter("mask_e", [128, Rb], F32, isOutput=False)
    mi_d = nc.declare_dram_parameter("mask_i", [128, Rb], F32, isOutput=False)
    mo_d = nc.declare_dram_parameter("mask_o", [128, Rs], F32, isOutput=False)
    xr_d = nc.declare_dram_parameter("xres", [128, Rb], F32, isOutput=False)
    yr_d = nc.declare_dram_parameter("yres", [128, Rs], F32, isOutput=False)
    out_d = nc.declare_dram_parameter("out", [128, 4], F32, isOutput=True)

    with ExitStack() as ctx:
        tc = ctx.enter_context(tile.TileContext(nc))
        singles = ctx.enter_context(tc.tile_pool(name="singles", bufs=1))
        d16p = ctx.enter_context(tc.tile_pool(name="d16p", bufs=4))
        l1p = ctx.enter_context(tc.tile_pool(name="l1p", bufs=2))
        l2p = ctx.enter_context(tc.tile_pool(name="l2p", bufs=2))
        l3p = ctx.enter_context(tc.tile_pool(name="l3p", bufs=2))

        RB = singles.tile([128, TOTBb, HB], BF16)
        WB = singles.tile([128, Rb, SC], BF16)
        RS = singles.tile([128, TOTBs, HB], BF16)
        WS = singles.tile([128, Rs, SC], BF16)
        RBf = RB.rearrange("p a b -> p (a b)")
        WBf = WB.rearrange("p a b -> p (a b)")
        RSf = RS.rearrange("p a b -> p (a b)")
        WSf = WS.rearrange("p a b -> p (a b)")
        # scalar queue must stay free for drains: bulk DMA on sync+gpsimd
        dq = [nc.sync, nc.gpsimd]
        for c in range(4):
            dq[c % 2].dma_start(out=WBf[32 * c:32 * c + KD, :], in_=wb_d[c])
            dq[(c + 1) % 2].dma_start(
                out=WSf[32 * c:32 * c + KD, :], in_=ws_d[c])
        # rhs cells, chunked; chunk0 of all classes first for fast start.
        # sync is HW-DGE (fast); gpsimd is SW-DGE (slow drains) - use 3:1.
        qi = [0]

        def rchunks(totb, RF, r_d, queues, early=0):
            chunks = [(0, min(8, totb))]
            lo = chunks[0][1]
            while lo < totb:
                hi = min(lo + 24, totb)
                chunks.append((lo, hi))
                lo = hi
            for k, (lo, hi) in enumerate(chunks):
                for c in range(4):
                    if k < early and c >= 2:
                        q = nc.scalar  # idle until first drain
                    else:
                        q = queues[qi[0] % len(queues)]
                        qi[0] += 1
                    q.dma_start(
                        out=RF[32 * c:32 * c + KD, lo * HB:hi * HB],
                        in_=r_d[c, :, lo * HB:hi * HB])

        rchunks(TOTBb, RBf, rb_d, [nc.sync, nc.gpsimd], early=2)
        rchunks(TOTBs, RSf, rs_d, [nc.scalar, nc.sync, nc.gpsimd, nc.sync])
        # prewarm ACT spline tables (sqrt/tanh) while DMAs land
        warm = singles.tile([128, 1], F32)
        nc.vector.memset(warm, 1.0)
        nc.scalar.sqrt(warm, warm)
        nc.scalar.activation(warm, warm, AF.Tanh, scale=1.0)

        me = singles.tile([128, Rb], F32)
        nc.scalar.dma_start(out=me, in_=me_d[:, :])
        mi = singles.tile([128, Rb], F32)
        nc.scalar.dma_start(out=mi, in_=mi_d[:, :])
        mo = singles.tile([128, Rs], F32)
        nc.scalar.dma_start(out=mo, in_=mo_d[:, :])
        xr = singles.tile([128, Rb], F32)
        nc.scalar.dma_start(out=xr, in_=xr_d[:, :])
        yr = singles.tile([128, Rs], F32)
        nc.scalar.dma_start(out=yr, in_=yr_d[:, :])

        MPB = singles.tile([128, TOTBb + 8], F32)  # per-half partial minima
        MPS = singles.tile([128, TOTBs + 8], F32)
        MHB = singles.tile([128, Rb], F32)  # per-rank minima
        MHS = singles.tile([128, Rs], F32)

        act_t = [0.0]
        dve_t = [0.0]

        def consume(ps, H, NHW, MP, h0):
            # dst: MP[:, h0:h0+NHW] per-half minima (+1 pad col if odd)
            nout = 2 * H
            dst = MP[:, h0:h0 + nout]
            aA = (512 * H + 352) / 1.2
            dA = (232 + 288 * H) / 0.96
            dC = (120 + 512 * H) / 0.96
            costA = max(act_t[0] + aA, dve_t[0] + dA)
            costC = max(act_t[0], dve_t[0] + dC)
            if costC < costA:
                psv = ps[:, 0:H, :].rearrange("p h (t x) -> p (h t) x", t=2)
                nc.vector.tensor_reduce(dst, psv, axis=AX, op=MIN)
                dve_t[0] += dC
                return
            d16 = d16p.tile([128, 4, BK], F16)
            nc.scalar.copy(d16[:, 0:H, :], ps[:, 0:H, :])
            l1 = l1p.tile([128, 4, 2, BK // 4], F16)
            d16v = d16.rearrange("p h (t x) -> p h t x", t=2)
            nc.vector.tensor_tensor(
                l1[:, 0:H, :, :], d16v[:, 0:H, :, 0:BK // 4],
                d16v[:, 0:H, :, BK // 4:BK // 2], MIN)
            l2 = l2p.tile([128, 4, 2, BK // 8], F16)
            nc.vector.tensor_tensor(
                l2[:, 0:H, :, :], l1[:, 0:H, :, 0:BK // 8],
                l1[:, 0:H, :, BK // 8:BK // 4], MIN)
            l3 = l3p.tile([128, 4, 2, BK // 16], F16)
            nc.vector.tensor_tensor(
                l3[:, 0:H, :, :], l2[:, 0:H, :, 0:BK // 16],
                l2[:, 0:H, :, BK // 16:BK // 8], MIN)
            l3v = l3.rearrange("p h t x -> p (h t) x")
            nc.vector.tensor_reduce(dst, l3v[:, 0:nout, :], axis=AX, op=MIN)
            act_t[0] += aA
            dve_t[0] += dA

        def run_pass(TOTH, half_rank, RT, WT, MP, MH, ranges):
            # halves of 256 cols; wave = up to 8 halves = 4 PSUM banks
            nw = (TOTH + 7) // 8
            with tc.tile_pool(name="ps", bufs=2, space="PSUM") as psp:
                for w in range(nw):
                    h0 = 8 * w
                    NHW = min(8, TOTH - h0)
                    H = (NHW + 1) // 2
                    ps = psp.tile([128, 4, BK], F32)
                    for c in range(4):
                        for j in range(NHW):
                            r = half_rank[h0 + j]
                            nc.tensor.matmul(
                                ps[32 * c:32 * c + 32, j // 2,
                                   (j % 2) * HB:(j % 2) * HB + HB],
                                WT[32 * c:32 * c + KD, r * SC:(r + 1) * SC],
                                RT[32 * c:32 * c + KD,
                                   (h0 + j) * HB:(h0 + j + 1) * HB],
                                start=True, stop=True,
                                tile_position=(32 * c, 32 * c),
                            )
                        if NHW % 2 == 1:
                            # fill odd tail with a repeat (keeps reduce valid)
                            r = half_rank[h0 + NHW - 1]
                            nc.tensor.matmul(
                                ps[32 * c:32 * c + 32, NHW // 2, HB:2 * HB],
                                WT[32 * c:32 * c + KD, r * SC:(r + 1) * SC],
                                RT[32 * c:32 * c + KD,
                                   (h0 + NHW - 1) * HB:(h0 + NHW) * HB],
                                start=True, stop=True,
                                tile_position=(32 * c, 32 * c),
                            )
                    consume(ps, H, NHW, MP, h0)
                    # emit rank reduces whose halves completed in this wave
                    for r, (lo, hi) in enumerate(ranges):
                        if h0 < hi <= h0 + NHW:
                            nc.vector.tensor_reduce(
                                MH[:, r:r + 1], MP[:, lo:hi], axis=AX, op=MIN)

        outsb = singles.tile([128, 4], F32)
        jh = singles.tile([128, Rb], F32)
        jh2 = singles.tile([128, Rb], F32)
        jo = singles.tile([128, Rs], F32)
        run_pass(TOTBb, plan["bank_rank_big"], RBf, WBf, MPB, MHB,
                 plan["ranges_big"])
        run_pass(TOTBs, plan["bank_rank_small"], RSf, WSf, MPS, MHS,
                 plan["ranges_small"])
        # end phase, batched by ACT table set
        nc.vector.tensor_tensor(MHB, MHB, xr, ADD)
        nc.vector.tensor_tensor(MHS, MHS, yr, ADD)
        for MH in (MHB, MHS):
            nc.vector.tensor_scalar_max(MH, MH, 0.0)
            nc.vector.tensor_scalar_min(MH, MH, 1.0e4)
        nc.scalar.sqrt(MHB, MHB)
        nc.scalar.sqrt(MHS, MHS)
        nc.scalar.activation(MHB, MHB, AF.Tanh, scale=1.0 / (0.025 * float(CS)))
        nc.scalar.activation(MHS, MHS, AF.Tanh, scale=1.0 / (0.025 * float(CS)))
        nc.vector.tensor_tensor(jh, MHB, me, MULT)
        nc.vector.tensor_reduce(outsb[:, 0:1], jh, axis=AX, op=ADD)
        nc.vector.tensor_tensor(jh2, MHB, mi, MULT)
        nc.vector.tensor_reduce(outsb[:, 1:2], jh2, axis=AX, op=ADD)
        nc.vector.tensor_tensor(jo, MHS, mo, MULT)
        nc.vector.tensor_reduce(outsb[:, 2:3], jo, axis=AX, op=ADD)
        nc.vector.memset(outsb[:, 3:4], 0.0)
        nc.sync.dma_start(out=out_d[:, :], in_=outsb)
    nc.compile()
    return nc


def _get_nc(plan):
    key = plan["sig"]
    if key not in _nc_cache:
        _nc_cache[key] = _build(plan)
    return _nc_cache[key]


# ---------------------------------------------------------------- kernel
def kernel(hand_verts, obj_verts, obj_split_sizes, exterior_hand, exterior_obj):
    import ml_dtypes

    hv = np.ascontiguousarray(hand_verts, dtype=np.float32) * CS
    ov = np.ascontiguousarray(obj_verts, dtype=np.float32) * CS
    splits = np.asarray(obj_split_sizes).astype(np.int64).reshape(B)
    eh = np.asarray(exterior_hand).astype(bool).reshape(B, NH)
    eo = np.asarray(exterior_obj).astype(bool).reshape(B, NO)
    valid = np.arange(NO)[None, :] < splits[:, None]
    interior = (~eo) & valid

    xx = (hv * hv).sum(-1).astype(np.float32)
    yy = (ov * ov).sum(-1).astype(np.float32)

    def split3(x):
        x0 = x.astype(ml_dtypes.bfloat16).astype(np.float32)
        r = x - x0
        x1 = r.astype(ml_dtypes.bfloat16).astype(np.float32)
        return x0, x1, r - x1

    o0, o1, o2 = split3(ov)
    h0, h1, h2 = split3(hv)
    y0, y1, y2 = split3(yy)
    x0, x1, x2 = split3(xx)
    xres = x1 + x2  # per-hand-vert residual, added post-min
    yres = y1 + y2  # per-obj-vert residual, added post-min
    A_SEQ = [o0, o0, o1, o1, o0, o2]
    B_SEQ = [h0, h1, h0, h1, h2, h0]

    # ---------- per-batch geometry plans ----------
    plans = []
    for b in range(B):
        v = int(splits[b])
        hvd = hv[b].astype(np.float64) / float(CS)
        ovd = ov[b, :v].astype(np.float64) / float(CS)
        hord = _kd_order(hvd, SC)
        oord = _kd_order(ovd, FB)
        hs = hvd[hord]
        os_ = ovd[oord]
        big_need = _needed_blocks(hs, os_, os_[::4])
        io_mask = interior[b, :v][oord]
        iidx = np.nonzero(io_mask)[0]
        ipts = os_[iidx]
        iord2 = _kd_order(ipts, SC)
        ipts = ipts[iord2]
        iglob = oord[iidx[iord2]]
        small_need = _needed_blocks(ipts, hs, hs[::2])
        nbk_big = [max(1, (len(n) + BPH - 1) // BPH) for n in big_need]
        nbk_small = [max(1, (len(n) + BPH - 1) // BPH) for n in small_need]
        plans.append({
            "v": v, "hord": hord, "oord": oord, "iglob": iglob,
            "big_need": big_need, "small_need": small_need,
            "nbk_big": nbk_big, "nbk_small": nbk_small,
            "load": sum(nbk_big) + sum(nbk_small),
        })

    # ---------- batch -> (core, slot) snake by load ----------
    order = np.argsort([-plans[b]["load"] for b in range(B)], kind="stable")
    slot_batches = np.empty((BPC, NCORES), np.int64)
    for s in range(BPC):
        seg = order[s * NCORES:(s + 1) * NCORES]
        if s % 2 == 1:
            seg = seg[::-1]
        slot_batches[s] = seg

    # ---------- unified rank-pure schedule ----------
    def unify(pass_key, nranks_per_slot):
        # Snake-deal sub-chunks (sorted desc by bank count) into the 4
        # classes so every class's rank-r count is as similar as possible.
        rank_maps = {}
        nbk = []
        for s in range(BPC):
            nr = nranks_per_slot[s]
            ranks = [1] * nr
            for core in range(NCORES):
                p = plans[slot_batches[s, core]]
                counts = p[pass_key]
                order_q = sorted(range(len(counts)), key=lambda q: -counts[q])
                percls = [[] for _ in range(4)]
                for r, q in enumerate(order_q):
                    c = r % 4 if (r // 4) % 2 == 0 else 3 - r % 4
                    percls[c].append(q)
                for c in range(4):
                    rank_maps[(core, s, c)] = percls[c]
                    for r, q in enumerate(percls[c]):
                        if r < nr:
                            ranks[r] = max(ranks[r], counts[q])
            nbk.append(ranks)
        bank_rank = []
        ranges = []
        slot_rank_base = []
        rglob = 0
        for s in range(BPC):
            slot_rank_base.append(rglob)
            for r, nb in enumerate(nbk[s]):
                lo = len(bank_rank)
                bank_rank.extend([rglob] * nb)
                ranges.append((lo, lo + nb))
                rglob += 1
        return (rank_maps, nbk, bank_rank, ranges, len(bank_rank), rglob,
                slot_rank_base)

    nranks_b = [max(len(range(c, NSC, 4)) for c in range(4))
                for s in range(BPC)]  # 7
    (rmap_b, nbk_b, bank_rank_b, ranges_b, TOTBb, Rb, srb_b) = \
        unify("nbk_big", nranks_b)

    nranks_s = []
    for s in range(BPC):
        mx = 1
        for core in range(NCORES):
            p = plans[slot_batches[s, core]]
            for c in range(4):
                mx = max(mx, len(range(c, len(p["nbk_small"]), 4)))
        nranks_s.append(mx)
    (rmap_s, nbk_s, bank_rank_s, ranges_s, TOTBs, Rs, srb_s) = \
        unify("nbk_small", nranks_s)

    plan = {
        "totb_big": TOTBb, "totb_small": TOTBs,
        "r_big": Rb, "r_small": Rs,
        "bank_rank_big": bank_rank_b, "bank_rank_small": bank_rank_s,
        "ranges_big": ranges_b, "ranges_small": ranges_s,
    }
    plan["sig"] = repr((TOTBb, TOTBs, Rb, Rs, bank_rank_b, bank_rank_s))

    # ---------- per-core packing ----------
    in_maps = []
    for core in range(NCORES):
        rb = np.zeros((4, KD, TOTBb * HB), np.float32)
        wb = np.zeros((4, KD, Rb * SC), np.float32)
        rs = np.zeros((4, KD, TOTBs * HB), np.float32)
        ws = np.zeros((4, KD, Rs * SC), np.float32)
        me = np.zeros((128, Rb), np.float32)
        mi = np.zeros((128, Rb), np.float32)
        mo = np.zeros((128, Rs), np.float32)
        xr = np.zeros((128, Rb), np.float32)
        yr = np.zeros((128, Rs), np.float32)
        for s in range(BPC):
            b = int(slot_batches[s, core])
            p = plans[b]
            v = p["v"]
            hord, oord = p["hord"], p["oord"]
            nop = ((v + FB - 1) // FB) * FB
            # big pass: rhs = obj cols, lhsT = hand lanes
            ocols = np.zeros((KD, nop), np.float32)
            for j in range(6):
                for d in range(3):
                    ocols[3 * j + d, :v] = A_SEQ[j][b, oord, d]
            ocols[18, :v] = y0[b, oord]
            ocols[18, v:] = BIG
            ocols[19, :v] = y1[b, oord]
            ocols[20, :v] = y2[b, oord]
            ocols[21, :v] = 1.0
            hlanes = np.zeros((KD, NHP), np.float32)
            for j in range(6):
                for d in range(3):
                    hlanes[3 * j + d, :NH] = -2.0 * B_SEQ[j][b, hord, d]
            hlanes[18:21, :NH] = 1.0
            hlanes[21, :NH] = x0[b, hord]
            ehb = eh[b][hord]
            xresb = xres[b, hord]

            # small pass: rhs = hand cols, lhsT = interior obj lanes
            hcols = np.zeros((KD, NHP), np.float32)
            for j in range(6):
                for d in range(3):
                    hcols[3 * j + d, :NH] = -2.0 * B_SEQ[j][b, hord, d]
            hcols[18, :NH] = x0[b, hord]
            hcols[18, NH:] = PAD
            hcols[19, :NH] = x1[b, hord]
            hcols[20, :NH] = x2[b, hord]
            hcols[21, :NH] = 1.0
            iglob = p["iglob"]
            ni = len(iglob)
            olanes = np.zeros((KD, max(ni, 1)), np.float32)
            for j in range(6):
                for d in range(3):
                    olanes[3 * j + d, :ni] = A_SEQ[j][b, iglob, d]
            olanes[18:21, :ni] = 1.0
            olanes[21, :ni] = y0[b, iglob]
            yresb = yres[b, iglob]

            def pack(rank_maps, nbk_slot, rbase, ranges_of, need_key, cols,
                     lanes, nlanes_of, rarr, warr, mask_sinks):
                for c in range(4):
                    subs = rank_maps[(core, s, c)]
                    for r in range(len(nbk_slot)):
                        rg = rbase + r
                        nbu = nbk_slot[r]
                        lo = ranges_of[rg][0]
                        if r < len(subs):
                            q = subs[r]
                            nl = nlanes_of(q)
                            warr[c, :, rg * SC:rg * SC + nl] = \
                                lanes[:, q * SC:q * SC + nl]
                            blocks = p[need_key][q]
                            nblk = len(blocks)
                            tot = nbu * BPH
                            gath = [blocks[j % nblk] for j in range(tot)]
                            gcols = np.concatenate(
                                [np.arange(g * FB, (g + 1) * FB)
                                 for g in gath])
                            rarr[c, :, lo * HB:lo * HB + tot * FB] = \
                                cols[:, gcols]
                            for msk, vals in mask_sinks(q, nl):
                                msk[32 * c:32 * c + nl, rg] = vals
                        else:
                            rarr[c, :, lo * HB:(lo + nbu) * HB] = \
                                np.tile(cols[:, 0:HB], (1, nbu))

            pack(rmap_b, nbk_b[s], srb_b[s], ranges_b, "big_need", ocols,
                 hlanes, lambda q: min(SC, NH - q * SC), rb, wb,
                 lambda q, nl: [(me, ehb[q * SC:q * SC + nl]),
                                (mi, ~ehb[q * SC:q * SC + nl]),
                                (xr, xresb[q * SC:q * SC + nl])])
            pack(rmap_s, nbk_s[s], srb_s[s], ranges_s, "small_need", hcols,
                 olanes, lambda q: min(SC, ni - q * SC), rs, ws,
                 lambda q, nl: [(mo, 1.0),
                                (yr, yresb[q * SC:q * SC + nl])])

        in_maps.append({
            "rb": rb.astype(ml_dtypes.bfloat16),
            "wb": wb.astype(ml_dtypes.bfloat16),
            "rs": rs.astype(ml_dtypes.bfloat16),
            "ws": ws.astype(ml_dtypes.bfloat16),
            "mask_e": me, "mask_i": mi, "mask_o": mo,
            "xres": xr, "yres": yr,
        })

    nc = _get_nc(plan)
    res = run_bass_kernel_spmd(nc, in_maps, list(range(NCORES))).results

    nums = np.zeros(3, np.float64)
    for rr in res:
        nums += rr["out"][:, 0:3].astype(np.float64).sum(axis=0)
    dens = np.array([eh.sum(), (~eh).sum(), interior.sum()], dtype=np.float64)
    out = np.where(dens > 0, 0.025 * nums / np.maximum(dens, 1.0), 0.0)
    return out.astype(np.float32)


# revision 27
# speedup vs baseline: 1.0457x; 1.0457x over previous
"""ContactLoss Trainium2 kernel v3.2 (8 NeuronCores, batch data-parallel,
spatially-pruned KNN).

Big pass (minho): hand verts kd-sorted into sub-chunks of 32; valid obj
verts kd-sorted into 32-col blocks. Host computes exact lower bounds
(point-to-block-bbox) and upper bounds (dist to obj reps) and keeps, per
sub-chunk, only the obj blocks that can contain a nearest neighbour.
Needed blocks are host-gathered into dense 512-col banks. On device,
16-way PE tiling (32x32 diagonal tiles, tile_position=(32c,32c))
computes 4 sub-chunks (one per 32-lane class) concurrently. Schedule is
rank-pure: rank r = the r-th-largest sub-chunk of each class (sorted by
bank count); nbk[slot][r] = max bank count over classes and cores, so a
single SPMD program serves all 8 cores. Wave = 4 consecutive banks; one
[22,32]x[22,512] matmul per (bank, class); consume reduces the wave to
per-bank minima columns of MP; one tiny contiguous reduce per rank
collapses its banks into MH, then masked sums produce the three loss
numerators. Small pass (minoh for interior obj): same machinery, roles
swapped. Classes pad short ranks by repeating their own blocks
(min-safe).

d2 precision: xx+yy-2xy with bf16 split products (6 pairs). The lhsT
side carries only the leading bf16 term of its squared-norm (x0 big /
y0 small); the residual (x1+x2 or y1+y2) is constant per output lane,
so it is added after the min on-device (keeps the f16 drain centred
near zero). COORD_SCALE keeps f16 drains in range.
"""

import sys
from contextlib import ExitStack

import numpy as np

sys.path.insert(0, "/opt/trn_rl_repo")

import concourse.mybir as mybir  # noqa: E402
import concourse.tile as tile  # noqa: E402
from concourse import bacc  # noqa: E402
from concourse.bass_utils import run_bass_kernel_spmd  # noqa: E402

B, NH, NO = 32, 778, 8192
NCORES = 8
BPC = B // NCORES
SC = 32  # sub-chunk lanes
NSC = (NH + SC - 1) // SC  # 25 hand sub-chunks
FB = 16  # fine block cols for the need test / gather granularity
BK = 512  # PSUM bank columns
HB = 256  # half-bank columns (scheduling granularity)
BPH = HB // FB  # fine blocks per half-bank
KD = 22
CS = np.float32(16.0)
BIG = np.float32(49152.0)
PAD = np.float32(8192.0)
NHP = ((NH + FB - 1) // FB) * FB  # 800

F32 = mybir.dt.float32
F16 = mybir.dt.float16
BF16 = mybir.dt.bfloat16
MIN = mybir.AluOpType.min
MULT = mybir.AluOpType.mult
ADD = mybir.AluOpType.add
AX = mybir.AxisListType.X
AF = mybir.ActivationFunctionType

_nc_cache = {}


# ---------------------------------------------------------------- geometry
def _kd_order(pts, leaf):
    """Recursive median split on widest axis -> contiguous leaves."""
    out = []

    def rec(ids):
        if len(ids) <= leaf:
            out.append(ids)
            return
        p = pts[ids]
        ax = int(np.argmax(p.max(0) - p.min(0)))
        k = len(ids) // 2
        part = np.argpartition(p[:, ax], k)
        rec(ids[part[:k]])
        rec(ids[part[k:]])

    sys.setrecursionlimit(100000)
    rec(np.arange(len(pts)))
    return np.concatenate(out)


def _box_dist2(q, blo, bhi):
    d = np.maximum(np.maximum(blo[None, :, :] - q[:, None, :],
                              q[:, None, :] - bhi[None, :, :]), 0.0)
    return (d * d).sum(-1)


def _needed_blocks(lanes_pts, sorted_pts, reps):
    """Per sub-chunk-of-32 of lanes_pts: indices of needed FB-blocks."""
    n = len(sorted_pts)
    starts = np.arange(0, n, FB)
    blo = np.minimum.reduceat(sorted_pts, starts)
    bhi = np.maximum.reduceat(sorted_pts, starts)
    d2 = ((lanes_pts[:, None, :] - reps[None, :, :]) ** 2).sum(-1)
    u2 = d2.min(1)
    lb2 = _box_dist2(lanes_pts, blo, bhi)
    need = lb2 <= u2[:, None] * (1.0 + 1e-9) + 1e-12
    nsub = (len(lanes_pts) + SC - 1) // SC
    return [np.nonzero(need[s * SC:(s + 1) * SC].any(0))[0]
            for s in range(nsub)]


# ---------------------------------------------------------------- device
def _build(plan):
    nc = bacc.Bacc("TRN2", target_bir_lowering=False, debug=False,
                   num_devices=NCORES)
    TOTBb, TOTBs = plan["totb_big"], plan["totb_small"]
    Rb, Rs = plan["r_big"], plan["r_small"]

    rb_d = nc.declare_dram_parameter("rb", [4, KD, TOTBb * HB], BF16,
                                     isOutput=False)
    wb_d = nc.declare_dram_parameter("wb", [4, KD, Rb * SC], BF16,
                                     isOutput=False)
    rs_d = nc.declare_dram_parameter("rs", [4, KD, TOTBs * HB], BF16,
                                     isOutput=False)
    ws_d = nc.declare_dram_parameter("ws", [4, KD, Rs * SC], BF16,
                                     isOutput=False)
    me_d = nc.declare_dram_parameter("mask_e", [128, Rb], F32, isOutput=False)
    mi_d = nc.declare_dram_parameter("mask_i", [128, Rb], F32, isOutput=False)
    mo_d = nc.declare_dram_parameter("mask_o", [128, Rs], F32, isOutput=False)
    xr_d = nc.declare_dram_parameter("xres", [128, Rb], F32, isOutput=False)
    yr_d = nc.declare_dram_parameter("yres", [128, Rs], F32, isOutput=False)
    out_d = nc.declare_dram_parameter("out", [128, 4], F32, isOutput=True)

    with ExitStack() as ctx:
        tc = ctx.enter_context(tile.TileContext(nc))
        singles = ctx.enter_context(tc.tile_pool(name="singles", bufs=1))
        d16p = ctx.enter_context(tc.tile_pool(name="d16p", bufs=4))
        l1p = ctx.enter_context(tc.tile_pool(name="l1p", bufs=2))
        l2p = ctx.enter_context(tc.tile_pool(name="l2p", bufs=2))
        l3p = ctx.enter_context(tc.tile_pool(name="l3p", bufs=2))

        RB = singles.tile([128, TOTBb, HB], BF16)
        WB = singles.tile([128, Rb, SC], BF16)
        RS = singles.tile([128, TOTBs, HB], BF16)
        WS = singles.tile([128, Rs, SC], BF16)
        RBf = RB.rearrange("p a b -> p (a b)")
        WBf = WB.rearrange("p a b -> p (a b)")
        RSf = RS.rearrange("p a b -> p (a b)")
        WSf = WS.rearrange("p a b -> p (a b)")
        # scalar queue must stay free for drains: bulk DMA on sync+gpsimd
        dq = [nc.sync, nc.gpsimd]
        for c in range(4):
            dq[c % 2].dma_start(out=WBf[32 * c:32 * c + KD, :], in_=wb_d[c])
            dq[(c + 1) % 2].dma_start(
                out=WSf[32 * c:32 * c + KD, :], in_=ws_d[c])
        # rhs cells, chunked; chunk0 of all classes first for fast start.
        # sync is HW-DGE (fast); gpsimd is SW-DGE (slow drains) - use 3:1.
        qi = [0]

        def rchunks(totb, RF, r_d, queues, early=0):
            chunks = [(0, min(8, totb))]
            lo = chunks[0][1]
            while lo < totb:
                hi = min(lo + 24, totb)
                chunks.append((lo, hi))
                lo = hi
            for k, (lo, hi) in enumerate(chunks):
                for c in range(4):
                    if k < early and c >= 2:
                        q = nc.scalar  # idle until first drain
                    else:
                        q = queues[qi[0] % len(queues)]
                        qi[0] += 1
                    q.dma_start(
                        out=RF[32 * c:32 * c + KD, lo * HB:hi * HB],
                        in_=r_d[c, :, lo * HB:hi * HB])

        rchunks(TOTBb, RBf, rb_d, [nc.sync, nc.gpsimd], early=2)
        rchunks(TOTBs, RSf, rs_d, [nc.scalar, nc.sync, nc.gpsimd, nc.sync])
        # prewarm ACT spline tables (sqrt/tanh) while DMAs land
        warm = singles.tile([128, 1], F32)
        nc.vector.memset(warm, 1.0)
        nc.scalar.sqrt(warm, warm)
        nc.scalar.activation(warm, warm, AF.Tanh, scale=1.0)

        me = singles.tile([128, Rb], F32)
        nc.scalar.dma_start(out=me, in_=me_d[:, :])
        mi = singles.tile([128, Rb], F32)
        nc.scalar.dma_start(out=mi, in_=mi_d[:, :])
        mo = singles.tile([128, Rs], F32)
        nc.scalar.dma_start(out=mo, in_=mo_d[:, :])
        xr = singles.tile([128, Rb], F32)
        nc.scalar.dma_start(out=xr, in_=xr_d[:, :])
        yr = singles.tile([128, Rs], F32)
        nc.scalar.dma_start(out=yr, in_=yr_d[:, :])

        MPB = singles.tile([128, TOTBb + 8], F32)  # per-half partial minima
        MPS = singles.tile([128, TOTBs + 8], F32)
        MHB = singles.tile([128, Rb], F32)  # per-rank minima
        MHS = singles.tile([128, Rs], F32)

        act_t = [0.0]
        dve_t = [0.0]

        def consume(ps, H, NHW, MP, h0):
            # dst: MP[:, h0:h0+NHW] per-half minima (+1 pad col if odd)
            nout = 2 * H
            dst = MP[:, h0:h0 + nout]
            aA = (512 * H + 352) / 1.2
            dA = (232 + 288 * H) / 0.96
            dC = (120 + 512 * H) / 0.96
            costA = max(act_t[0] + aA, dve_t[0] + dA)
            costC = max(act_t[0], dve_t[0] + dC)
            if costC < costA:
                psv = ps[:, 0:H, :].rearrange("p h (t x) -> p (h t) x", t=2)
                nc.vector.tensor_reduce(dst, psv, axis=AX, op=MIN)
                dve_t[0] += dC
                return
            d16 = d16p.tile([128, 4, BK], F16)
            nc.scalar.copy(d16[:, 0:H, :], ps[:, 0:H, :])
            l1 = l1p.tile([128, 4, 2, BK // 4], F16)
            d16v = d16.rearrange("p h (t x) -> p h t x", t=2)
            nc.vector.tensor_tensor(
                l1[:, 0:H, :, :], d16v[:, 0:H, :, 0:BK // 4],
                d16v[:, 0:H, :, BK // 4:BK // 2], MIN)
            l2 = l2p.tile([128, 4, 2, BK // 8], F16)
            nc.vector.tensor_tensor(
                l2[:, 0:H, :, :], l1[:, 0:H, :, 0:BK // 8],
                l1[:, 0:H, :, BK // 8:BK // 4], MIN)
            l3 = l3p.tile([128, 4, 2, BK // 16], F16)
            nc.vector.tensor_tensor(
                l3[:, 0:H, :, :], l2[:, 0:H, :, 0:BK // 16],
                l2[:, 0:H, :, BK // 16:BK // 8], MIN)
            l3v = l3.rearrange("p h t x -> p (h t) x")
            nc.vector.tensor_reduce(dst, l3v[:, 0:nout, :], axis=AX, op=MIN)
            act_t[0] += aA
            dve_t[0] += dA

        def run_pass(TOTH, half_rank, RT, WT, MP, MH, ranges):
            # halves of 256 cols; wave = up to 8 halves = 4 PSUM banks
            nw = (TOTH + 7) // 8
            with tc.tile_pool(name="ps", bufs=2, space="PSUM") as psp:
                for w in range(nw):
                    h0 = 8 * w
                    NHW = min(8, TOTH - h0)
                    H = (NHW + 1) // 2
                    ps = psp.tile([128, 4, BK], F32)
                    for c in range(4):
                        for j in range(NHW):
                            r = half_rank[h0 + j]
                            nc.tensor.matmul(
                                ps[32 * c:32 * c + 32, j // 2,
                                   (j % 2) * HB:(j % 2) * HB + HB],
                                WT[32 * c:32 * c + KD, r * SC:(r + 1) * SC],
                                RT[32 * c:32 * c + KD,
                                   (h0 + j) * HB:(h0 + j + 1) * HB],
                                start=True, stop=True,
                                tile_position=(32 * c, 32 * c),
                            )
                        if NHW % 2 == 1:
                            # fill odd tail with a repeat (keeps reduce valid)
                            r = half_rank[h0 + NHW - 1]
                            nc.tensor.matmul(
                                ps[32 * c:32 * c + 32, NHW // 2, HB:2 * HB],
                                WT[32 * c:32 * c + KD, r * SC:(r + 1) * SC],
                                RT[32 * c:32 * c + KD,
                                   (h0 + NHW - 1) * HB:(h0 + NHW) * HB],
                                start=True, stop=True,
                                tile_position=(32 * c, 32 * c),
                            )
                    consume(ps, H, NHW, MP, h0)
                    # emit rank reduces whose halves completed in this wave
                    for r, (lo, hi) in enumerate(ranges):
                        if h0 < hi <= h0 + NHW:
                            nc.vector.tensor_reduce(
                                MH[:, r:r + 1], MP[:, lo:hi], axis=AX, op=MIN)

        outsb = singles.tile([128, 4], F32)
        jh = singles.tile([128, Rb], F32)
        jh2 = singles.tile([128, Rb], F32)
        jo = singles.tile([128, Rs], F32)
        run_pass(TOTBb, plan["bank_rank_big"], RBf, WBf, MPB, MHB,
                 plan["ranges_big"])
        run_pass(TOTBs, plan["bank_rank_small"], RSf, WSf, MPS, MHS,
                 plan["ranges_small"])
        # end phase, batched by ACT table set
        nc.vector.tensor_tensor(MHB, MHB, xr, ADD)
        nc.vector.tensor_tensor(MHS, MHS, yr, ADD)
        for MH in (MHB, MHS):
            nc.vector.tensor_scalar_max(MH, MH, 0.0)
            nc.vector.tensor_scalar_min(MH, MH, 1.0e4)
        nc.scalar.sqrt(MHB, MHB)
        nc.scalar.sqrt(MHS, MHS)
        nc.scalar.activation(MHB, MHB, AF.Tanh, scale=1.0 / (0.025 * float(CS)))
        nc.scalar.activation(MHS, MHS, AF.Tanh, scale=1.0 / (0.025 * float(CS)))
        nc.vector.tensor_tensor(jh, MHB, me, MULT)
        nc.vector.tensor_reduce(outsb[:, 0:1], jh, axis=AX, op=ADD)
        nc.vector.tensor_tensor(jh2, MHB, mi, MULT)
        nc.vector.tensor_reduce(outsb[:, 1:2], jh2, axis=AX, op=ADD)
        nc.vector.tensor_tensor(jo, MHS, mo, MULT)
        nc.vector.tensor_reduce(outsb[:, 2:3], jo, axis=AX, op=ADD)
        nc.vector.memset(outsb[:, 3:4], 0.0)
        nc.sync.dma_start(out=out_d[:, :], in_=outsb)
    nc.compile()
    return nc


def _get_nc(plan):
    key = plan["sig"]
    if key not in _nc_cache:
        _nc_cache[key] = _build(plan)
    return _nc_cache[key]


# ---------------------------------------------------------------- kernel
def kernel(hand_verts, obj_verts, obj_split_sizes, exterior_hand, exterior_obj):
    import ml_dtypes

    hv = np.ascontiguousarray(hand_verts, dtype=np.float32) * CS
    ov = np.ascontiguousarray(obj_verts, dtype=np.float32) * CS
    splits = np.asarray(obj_split_sizes).astype(np.int64).reshape(B)
    eh = np.asarray(exterior_hand).astype(bool).reshape(B, NH)
    eo = np.asarray(exterior_obj).astype(bool).reshape(B, NO)
    valid = np.arange(NO)[None, :] < splits[:, None]
    interior = (~eo) & valid

    xx = (hv * hv).sum(-1).astype(np.float32)
    yy = (ov * ov).sum(-1).astype(np.float32)

    def split3(x):
        x0 = x.astype(ml_dtypes.bfloat16).astype(np.float32)
        r = x - x0
        x1 = r.astype(ml_dtypes.bfloat16).astype(np.float32)
        return x0, x1, r - x1

    o0, o1, o2 = split3(ov)
    h0, h1, h2 = split3(hv)
    y0, y1, y2 = split3(yy)
    x0, x1, x2 = split3(xx)
    xres = x1 + x2  # per-hand-vert residual, added post-min
    yres = y1 + y2  # per-obj-vert residual, added post-min
    A_SEQ = [o0, o0, o1, o1, o0, o2]
    B_SEQ = [h0, h1, h0, h1, h2, h0]

    # ---------- per-batch geometry plans ----------
    plans = []
    for b in range(B):
        v = int(splits[b])
        hvd = hv[b].astype(np.float64) / float(CS)
        ovd = ov[b, :v].astype(np.float64) / float(CS)
        hord = _kd_order(hvd, SC)
        oord = _kd_order(ovd, FB)
        hs = hvd[hord]
        os_ = ovd[oord]
        big_need = _needed_blocks(hs, os_, os_[::4])
        io_mask = interior[b, :v][oord]
        iidx = np.nonzero(io_mask)[0]
        ipts = os_[iidx]
        iord2 = _kd_order(ipts, SC)
        ipts = ipts[iord2]
        iglob = oord[iidx[iord2]]
        small_need = _needed_blocks(ipts, hs, hs[::2])
        nbk_big = [max(1, (len(n) + BPH - 1) // BPH) for n in big_need]
        nbk_small = [max(1, (len(n) + BPH - 1) // BPH) for n in small_need]
        plans.append({
            "v": v, "hord": hord, "oord": oord, "iglob": iglob,
            "big_need": big_need, "small_need": small_need,
            "nbk_big": nbk_big, "nbk_small": nbk_small,
            "load": sum(nbk_big) + sum(nbk_small),
        })

    # ---------- batch -> (core, slot) snake by load ----------
    order = np.argsort([-plans[b]["load"] for b in range(B)], kind="stable")
    slot_batches = np.empty((BPC, NCORES), np.int64)
    for s in range(BPC):
        seg = order[s * NCORES:(s + 1) * NCORES]
        if s % 2 == 1:
            seg = seg[::-1]
        slot_batches[s] = seg

    # ---------- unified rank-pure schedule ----------
    def unify(pass_key, nranks_per_slot):
        # Snake-deal sub-chunks (sorted desc by bank count) into the 4
        # classes so every class's rank-r count is as similar as possible.
        rank_maps = {}
        nbk = []
        for s in range(BPC):
            nr = nranks_per_slot[s]
            ranks = [1] * nr
            for core in range(NCORES):
                p = plans[slot_batches[s, core]]
                counts = p[pass_key]
                order_q = sorted(range(len(counts)), key=lambda q: -counts[q])
                percls = [[] for _ in range(4)]
                for r, q in enumerate(order_q):
                    c = r % 4 if (r // 4) % 2 == 0 else 3 - r % 4
                    percls[c].append(q)
                for c in range(4):
                    rank_maps[(core, s, c)] = percls[c]
                    for r, q in enumerate(percls[c]):
                        if r < nr:
                            ranks[r] = max(ranks[r], counts[q])
            nbk.append(ranks)
        bank_rank = []
        ranges = []
        slot_rank_base = []
        rglob = 0
        for s in range(BPC):
            slot_rank_base.append(rglob)
            for r, nb in enumerate(nbk[s]):
                lo = len(bank_rank)
                bank_rank.extend([rglob] * nb)
                ranges.append((lo, lo + nb))
                rglob += 1
        return (rank_maps, nbk, bank_rank, ranges, len(bank_rank), rglob,
                slot_rank_base)

    nranks_b = [max(len(range(c, NSC, 4)) for c in range(4))
                for s in range(BPC)]  # 7
    (rmap_b, nbk_b, bank_rank_b, ranges_b, TOTBb, Rb, srb_b) = \
        unify("nbk_big", nranks_b)

    nranks_s = []
    for s in range(BPC):
        mx = 1
        for core in range(NCORES):
            p = plans[slot_batches[s, core]]
            for c in range(4):
                mx = max(mx, len(range(c, len(p["nbk_small"]), 4)))
        nranks_s.append(mx)
    (rmap_s, nbk_s, bank_rank_s, ranges_s, TOTBs, Rs, srb_s) = \
        unify("nbk_small", nranks_s)

    plan = {
        "totb_big": TOTBb, "totb_small": TOTBs,
        "r_big": Rb, "r_small": Rs,
        "bank_rank_big": bank_rank_b, "bank_rank_small": bank_rank_s,
        "ranges_big": ranges_b, "ranges_small": ranges_s,
    }
    plan["sig"] = repr((TOTBb, TOTBs, Rb, Rs, bank_rank_b, bank_rank_s))

    # ---------- per-core packing ----------
    in_maps = []
    for core in range(NCORES):
        rb = np.zeros((4, KD, TOTBb * HB), np.float32)
        wb = np.zeros((4, KD, Rb * SC), np.float32)
        rs = np.zeros((4, KD, TOTBs * HB), np.float32)
        ws = np.zeros((4, KD, Rs * SC), np.float32)
        me = np.zeros((128, Rb), np.float32)
        mi = np.zeros((128, Rb), np.float32)
        mo = np.zeros((128, Rs), np.float32)
        xr = np.zeros((128, Rb), np.float32)
        yr = np.zeros((128, Rs), np.float32)
        for s in range(BPC):
            b = int(slot_batches[s, core])
            p = plans[b]
            v = p["v"]
            hord, oord = p["hord"], p["oord"]
            nop = ((v + FB - 1) // FB) * FB
            # big pass: rhs = obj cols, lhsT = hand lanes
            ocols = np.zeros((KD, nop), np.float32)
            for j in range(6):
                for d in range(3):
                    ocols[3 * j + d, :v] = A_SEQ[j][b, oord, d]
            ocols[18, :v] = y0[b, oord]
            ocols[18, v:] = BIG
            ocols[19, :v] = y1[b, oord]
            ocols[20, :v] = y2[b, oord]
            ocols[21, :v] = 1.0
            hlanes = np.zeros((KD, NHP), np.float32)
            for j in range(6):
                for d in range(3):
                    hlanes[3 * j + d, :NH] = -2.0 * B_SEQ[j][b, hord, d]
            hlanes[18:21, :NH] = 1.0
            hlanes[21, :NH] = x0[b, hord]
            ehb = eh[b][hord]
            xresb = xres[b, hord]

            # small pass: rhs = hand cols, lhsT = interior obj lanes
            hcols = np.zeros((KD, NHP), np.float32)
            for j in range(6):
                for d in range(3):
                    hcols[3 * j + d, :NH] = -2.0 * B_SEQ[j][b, hord, d]
            hcols[18, :NH] = x0[b, hord]
            hcols[18, NH:] = PAD
            hcols[19, :NH] = x1[b, hord]
            hcols[20, :NH] = x2[b, hord]
            hcols[21, :NH] = 1.0
            iglob = p["iglob"]
            ni = len(iglob)
            olanes = np.zeros((KD, max(ni, 1)), np.float32)
            for j in range(6):
                for d in range(3):
                    olanes[3 * j + d, :ni] = A_SEQ[j][b, iglob, d]
            olanes[18:21, :ni] = 1.0
            olanes[21, :ni] = y0[b, iglob]
            yresb = yres[b, iglob]

            def pack(rank_maps, nbk_slot, rbase, ranges_of, need_key, cols,
                     lanes, nlanes_of, rarr, warr, mask_sinks):
                for c in range(4):
                    subs = rank_maps[(core, s, c)]
                    for r in range(len(nbk_slot)):
                        rg = rbase + r
                        nbu = nbk_slot[r]
                        lo = ranges_of[rg][0]
                        if r < len(subs):
                            q = subs[r]
                            nl = nlanes_of(q)
                            warr[c, :, rg * SC:rg * SC + nl] = \
                                lanes[:, q * SC:q * SC + nl]
                            blocks = p[need_key][q]
                            nblk = len(blocks)
                            tot = nbu * BPH
                            gath = [blocks[j % nblk] for j in range(tot)]
                            gcols = np.concatenate(
                                [np.arange(g * FB, (g + 1) * FB)
                                 for g in gath])
                            rarr[c, :, lo * HB:lo * HB + tot * FB] = \
                                cols[:, gcols]
                            for msk, vals in mask_sinks(q, nl):
                                msk[32 * c:32 * c + nl, rg] = vals
                        else:
                            rarr[c, :, lo * HB:(lo + nbu) * HB] = \
                                np.tile(cols[:, 0:HB], (1, nbu))

            pack(rmap_b, nbk_b[s], srb_b[s], ranges_b, "big_need", ocols,
                 hlanes, lambda q: min(SC, NH - q * SC), rb, wb,
                 lambda q, nl: [(me, ehb[q * SC:q * SC + nl]),
                                (mi, ~ehb[q * SC:q * SC + nl]),
                                (xr, xresb[q * SC:q * SC + nl])])
            pack(rmap_s, nbk_s[s], srb_s[s], ranges_s, "small_need", hcols,
                 olanes, lambda q: min(SC, ni - q * SC), rs, ws,
                 lambda q, nl: [(mo, 1.0),
                                (yr, yresb[q * SC:q * SC + nl])])

        in_maps.append({
            "rb": rb.astype(ml_dtypes.bfloat16),
            "wb": wb.astype(ml_dtypes.bfloat16),
            "rs": rs.astype(ml_dtypes.bfloat16),
            "ws": ws.astype(ml_dtypes.bfloat16),
            "mask_e": me, "mask_i": mi, "mask_o": mo,
            "xres": xr, "yres": yr,
        })

    nc = _get_nc(plan)
    res = run_bass_kernel_spmd(nc, in_maps, list(range(NCORES))).results

    nums = np.zeros(3, np.float64)
    for rr in res:
        nums += rr["out"][:, 0:3].astype(np.float64).sum(axis=0)
    dens = np.array([eh.sum(), (~eh).sum(), interior.sum()], dtype=np.float64)
    out = np.where(dens > 0, 0.025 * nums / np.maximum(dens, 1.0), 0.0)
    return out.astype(np.float32)


# revision 28
# speedup vs baseline: 1.1510x; 1.1008x over previous
"""ContactLoss Trainium2 kernel v3.2 (8 NeuronCores, batch data-parallel,
spatially-pruned KNN).

Big pass (minho): hand verts kd-sorted into sub-chunks of 32; valid obj
verts kd-sorted into 32-col blocks. Host computes exact lower bounds
(point-to-block-bbox) and upper bounds (dist to obj reps) and keeps, per
sub-chunk, only the obj blocks that can contain a nearest neighbour.
Needed blocks are host-gathered into dense 512-col banks. On device,
16-way PE tiling (32x32 diagonal tiles, tile_position=(32c,32c))
computes 4 sub-chunks (one per 32-lane class) concurrently. Schedule is
rank-pure: rank r = the r-th-largest sub-chunk of each class (sorted by
bank count); nbk[slot][r] = max bank count over classes and cores, so a
single SPMD program serves all 8 cores. Wave = 4 consecutive banks; one
[22,32]x[22,512] matmul per (bank, class); consume reduces the wave to
per-bank minima columns of MP; one tiny contiguous reduce per rank
collapses its banks into MH, then masked sums produce the three loss
numerators. Small pass (minoh for interior obj): same machinery, roles
swapped. Classes pad short ranks by repeating their own blocks
(min-safe).

d2 precision: xx+yy-2xy with bf16 split products (6 pairs). The lhsT
side carries only the leading bf16 term of its squared-norm (x0 big /
y0 small); the residual (x1+x2 or y1+y2) is constant per output lane,
so it is added after the min on-device (keeps the f16 drain centred
near zero). COORD_SCALE keeps f16 drains in range.
"""

import sys
from contextlib import ExitStack

import numpy as np

sys.path.insert(0, "/opt/trn_rl_repo")

import concourse.mybir as mybir  # noqa: E402
import concourse.tile as tile  # noqa: E402
from concourse import bacc  # noqa: E402
from concourse.bass_utils import run_bass_kernel_spmd  # noqa: E402

B, NH, NO = 32, 778, 8192
NCORES = 8
BPC = B // NCORES
SC = 32  # sub-chunk lanes
NSC = (NH + SC - 1) // SC  # 25 hand sub-chunks
FB = 16  # fine block cols for the need test / gather granularity
BK = 512  # PSUM bank columns
HB = 256  # half-bank columns (scheduling granularity)
BPH = HB // FB  # fine blocks per half-bank
KD = 22
CS = np.float32(16.0)
BIG = np.float32(49152.0)
PAD = np.float32(8192.0)
NHP = ((NH + FB - 1) // FB) * FB  # 800

F32 = mybir.dt.float32
F16 = mybir.dt.float16
BF16 = mybir.dt.bfloat16
MIN = mybir.AluOpType.min
MULT = mybir.AluOpType.mult
ADD = mybir.AluOpType.add
AX = mybir.AxisListType.X
AF = mybir.ActivationFunctionType

_nc_cache = {}


# ---------------------------------------------------------------- geometry
def _kd_order(pts, leaf):
    """Recursive median split on widest axis -> contiguous leaves."""
    out = []

    def rec(ids):
        if len(ids) <= leaf:
            out.append(ids)
            return
        p = pts[ids]
        ax = int(np.argmax(p.max(0) - p.min(0)))
        k = len(ids) // 2
        part = np.argpartition(p[:, ax], k)
        rec(ids[part[:k]])
        rec(ids[part[k:]])

    sys.setrecursionlimit(100000)
    rec(np.arange(len(pts)))
    return np.concatenate(out)


def _box_dist2(q, blo, bhi):
    d = np.maximum(np.maximum(blo[None, :, :] - q[:, None, :],
                              q[:, None, :] - bhi[None, :, :]), 0.0)
    return (d * d).sum(-1)


def _needed_blocks(lanes_pts, sorted_pts, reps):
    """Per sub-chunk-of-32 of lanes_pts: indices of needed FB-blocks."""
    n = len(sorted_pts)
    starts = np.arange(0, n, FB)
    blo = np.minimum.reduceat(sorted_pts, starts)
    bhi = np.maximum.reduceat(sorted_pts, starts)
    d2 = ((lanes_pts[:, None, :] - reps[None, :, :]) ** 2).sum(-1)
    u2 = d2.min(1)
    lb2 = _box_dist2(lanes_pts, blo, bhi)
    need = lb2 <= u2[:, None] * (1.0 + 1e-9) + 1e-12
    nsub = (len(lanes_pts) + SC - 1) // SC
    return [np.nonzero(need[s * SC:(s + 1) * SC].any(0))[0]
            for s in range(nsub)]


# ---------------------------------------------------------------- device
def _build(plan):
    nc = bacc.Bacc("TRN2", target_bir_lowering=False, debug=False,
                   num_devices=NCORES)
    TOTBb, TOTBs = plan["totb_big"], plan["totb_small"]
    Rb, Rs = plan["r_big"], plan["r_small"]

    rb_d = nc.declare_dram_parameter("rb", [4, KD, TOTBb * HB], BF16,
                                     isOutput=False)
    wb_d = nc.declare_dram_parameter("wb", [4, KD, Rb * SC], BF16,
                                     isOutput=False)
    rs_d = nc.declare_dram_parameter("rs", [4, KD, TOTBs * HB], BF16,
                                     isOutput=False)
    ws_d = nc.declare_dram_parameter("ws", [4, KD, Rs * SC], BF16,
                                     isOutput=False)
    me_d = nc.declare_dram_parameter("mask_e", [128, Rb], F32, isOutput=False)
    mi_d = nc.declare_dram_parameter("mask_i", [128, Rb], F32, isOutput=False)
    mo_d = nc.declare_dram_parameter("mask_o", [128, Rs], F32, isOutput=False)
    xr_d = nc.declare_dram_parameter("xres", [128, Rb], F32, isOutput=False)
    yr_d = nc.declare_dram_parameter("yres", [128, Rs], F32, isOutput=False)
    out_d = nc.declare_dram_parameter("out", [128, 4], F32, isOutput=True)

    with ExitStack() as ctx:
        tc = ctx.enter_context(tile.TileContext(nc))
        singles = ctx.enter_context(tc.tile_pool(name="singles", bufs=1))
        d16p = ctx.enter_context(tc.tile_pool(name="d16p", bufs=4))
        l1p = ctx.enter_context(tc.tile_pool(name="l1p", bufs=2))
        l2p = ctx.enter_context(tc.tile_pool(name="l2p", bufs=2))
        l3p = ctx.enter_context(tc.tile_pool(name="l3p", bufs=2))

        RB = singles.tile([128, TOTBb, HB], BF16)
        WB = singles.tile([128, Rb, SC], BF16)
        RS = singles.tile([128, TOTBs, HB], BF16)
        WS = singles.tile([128, Rs, SC], BF16)
        RBf = RB.rearrange("p a b -> p (a b)")
        WBf = WB.rearrange("p a b -> p (a b)")
        RSf = RS.rearrange("p a b -> p (a b)")
        WSf = WS.rearrange("p a b -> p (a b)")
        # scalar queue must stay free for drains: bulk DMA on sync+gpsimd
        dq = [nc.sync, nc.gpsimd]
        for c in range(4):
            dq[c % 2].dma_start(out=WBf[32 * c:32 * c + KD, :], in_=wb_d[c])
            dq[(c + 1) % 2].dma_start(
                out=WSf[32 * c:32 * c + KD, :], in_=ws_d[c])
        # rhs cells, chunked; chunk0 of all classes first for fast start.
        # sync is HW-DGE (fast); gpsimd is SW-DGE (slow drains) - use 3:1.
        qi = [0]

        def rchunks(totb, RF, r_d, queues, early=0):
            chunks = [(0, min(8, totb))]
            lo = chunks[0][1]
            while lo < totb:
                hi = min(lo + 24, totb)
                chunks.append((lo, hi))
                lo = hi
            for k, (lo, hi) in enumerate(chunks):
                for c in range(4):
                    if k < early and c >= 2:
                        q = nc.scalar  # idle until first drain
                    else:
                        q = queues[qi[0] % len(queues)]
                        qi[0] += 1
                    q.dma_start(
                        out=RF[32 * c:32 * c + KD, lo * HB:hi * HB],
                        in_=r_d[c, :, lo * HB:hi * HB])

        rchunks(TOTBb, RBf, rb_d, [nc.sync, nc.gpsimd], early=2)
        rchunks(TOTBs, RSf, rs_d, [nc.scalar, nc.sync, nc.gpsimd, nc.sync])
        # prewarm ACT spline tables (sqrt/tanh) while DMAs land
        warm = singles.tile([128, 1], F32)
        nc.vector.memset(warm, 1.0)
        nc.scalar.sqrt(warm, warm)
        nc.scalar.activation(warm, warm, AF.Tanh, scale=1.0)

        me = singles.tile([128, Rb], F32)
        nc.scalar.dma_start(out=me, in_=me_d[:, :])
        mi = singles.tile([128, Rb], F32)
        nc.scalar.dma_start(out=mi, in_=mi_d[:, :])
        mo = singles.tile([128, Rs], F32)
        nc.scalar.dma_start(out=mo, in_=mo_d[:, :])
        xr = singles.tile([128, Rb], F32)
        nc.scalar.dma_start(out=xr, in_=xr_d[:, :])
        yr = singles.tile([128, Rs], F32)
        nc.scalar.dma_start(out=yr, in_=yr_d[:, :])

        MPB = singles.tile([128, TOTBb + 8], F32)  # per-half partial minima
        MPS = singles.tile([128, TOTBs + 8], F32)
        MHB = singles.tile([128, Rb], F32)  # per-rank minima
        MHS = singles.tile([128, Rs], F32)

        act_t = [0.0]
        dve_t = [0.0]

        def consume(ps, H, NHW, MP, h0):
            # dst: MP[:, h0:h0+NHW] per-half minima (+1 pad col if odd)
            nout = 2 * H
            dst = MP[:, h0:h0 + nout]
            aA = (512 * H + 352) / 1.2
            dA = (232 + 288 * H) / 0.96
            dC = (120 + 512 * H) / 0.96
            costA = max(act_t[0] + aA, dve_t[0] + dA)
            costC = max(act_t[0], dve_t[0] + dC)
            if costC < costA:
                psv = ps[:, 0:H, :].rearrange("p h (t x) -> p (h t) x", t=2)
                nc.vector.tensor_reduce(dst, psv, axis=AX, op=MIN)
                dve_t[0] += dC
                return
            d16 = d16p.tile([128, 4, BK], F16)
            nc.scalar.copy(d16[:, 0:H, :], ps[:, 0:H, :])
            l1 = l1p.tile([128, 4, 2, BK // 4], F16)
            d16v = d16.rearrange("p h (t x) -> p h t x", t=2)
            nc.vector.tensor_tensor(
                l1[:, 0:H, :, :], d16v[:, 0:H, :, 0:BK // 4],
                d16v[:, 0:H, :, BK // 4:BK // 2], MIN)
            l2 = l2p.tile([128, 4, 2, BK // 8], F16)
            nc.vector.tensor_tensor(
                l2[:, 0:H, :, :], l1[:, 0:H, :, 0:BK // 8],
                l1[:, 0:H, :, BK // 8:BK // 4], MIN)
            l3 = l3p.tile([128, 4, 2, BK // 16], F16)
            nc.vector.tensor_tensor(
                l3[:, 0:H, :, :], l2[:, 0:H, :, 0:BK // 16],
                l2[:, 0:H, :, BK // 16:BK // 8], MIN)
            l3v = l3.rearrange("p h t x -> p (h t) x")
            nc.vector.tensor_reduce(dst, l3v[:, 0:nout, :], axis=AX, op=MIN)
            act_t[0] += aA
            dve_t[0] += dA

        def run_pass(TOTH, half_rank, RT, WT, MP, MH, ranges):
            # halves of 256 cols; wave = up to 8 halves = 4 PSUM banks
            nw = (TOTH + 3) // 4
            with tc.tile_pool(name="ps", bufs=4, space="PSUM") as psp:
                for w in range(nw):
                    h0 = 4 * w
                    NHW = min(4, TOTH - h0)
                    H = (NHW + 1) // 2
                    ps = psp.tile([128, 2, BK], F32)
                    for c in range(4):
                        for j in range(NHW):
                            r = half_rank[h0 + j]
                            nc.tensor.matmul(
                                ps[32 * c:32 * c + 32, j // 2,
                                   (j % 2) * HB:(j % 2) * HB + HB],
                                WT[32 * c:32 * c + KD, r * SC:(r + 1) * SC],
                                RT[32 * c:32 * c + KD,
                                   (h0 + j) * HB:(h0 + j + 1) * HB],
                                start=True, stop=True,
                                tile_position=(32 * c, 32 * c),
                            )
                        if NHW % 2 == 1:
                            # fill odd tail with a repeat (keeps reduce valid)
                            r = half_rank[h0 + NHW - 1]
                            nc.tensor.matmul(
                                ps[32 * c:32 * c + 32, NHW // 2, HB:2 * HB],
                                WT[32 * c:32 * c + KD, r * SC:(r + 1) * SC],
                                RT[32 * c:32 * c + KD,
                                   (h0 + NHW - 1) * HB:(h0 + NHW) * HB],
                                start=True, stop=True,
                                tile_position=(32 * c, 32 * c),
                            )
                    consume(ps, H, NHW, MP, h0)
                    # emit rank reduces whose halves completed in this wave
                    for r, (lo, hi) in enumerate(ranges):
                        if h0 < hi <= h0 + NHW:
                            nc.vector.tensor_reduce(
                                MH[:, r:r + 1], MP[:, lo:hi], axis=AX, op=MIN)

        outsb = singles.tile([128, 4], F32)
        jh = singles.tile([128, Rb], F32)
        jh2 = singles.tile([128, Rb], F32)
        jo = singles.tile([128, Rs], F32)
        run_pass(TOTBb, plan["bank_rank_big"], RBf, WBf, MPB, MHB,
                 plan["ranges_big"])
        run_pass(TOTBs, plan["bank_rank_small"], RSf, WSf, MPS, MHS,
                 plan["ranges_small"])
        # end phase, batched by ACT table set
        nc.vector.tensor_tensor(MHB, MHB, xr, ADD)
        nc.vector.tensor_tensor(MHS, MHS, yr, ADD)
        for MH in (MHB, MHS):
            nc.vector.tensor_scalar_max(MH, MH, 0.0)
            nc.vector.tensor_scalar_min(MH, MH, 1.0e4)
        nc.scalar.sqrt(MHB, MHB)
        nc.scalar.sqrt(MHS, MHS)
        nc.scalar.activation(MHB, MHB, AF.Tanh, scale=1.0 / (0.025 * float(CS)))
        nc.scalar.activation(MHS, MHS, AF.Tanh, scale=1.0 / (0.025 * float(CS)))
        nc.vector.tensor_tensor(jh, MHB, me, MULT)
        nc.vector.tensor_reduce(outsb[:, 0:1], jh, axis=AX, op=ADD)
        nc.vector.tensor_tensor(jh2, MHB, mi, MULT)
        nc.vector.tensor_reduce(outsb[:, 1:2], jh2, axis=AX, op=ADD)
        nc.vector.tensor_tensor(jo, MHS, mo, MULT)
        nc.vector.tensor_reduce(outsb[:, 2:3], jo, axis=AX, op=ADD)
        nc.vector.memset(outsb[:, 3:4], 0.0)
        nc.sync.dma_start(out=out_d[:, :], in_=outsb)
    nc.compile()
    return nc


def _get_nc(plan):
    key = plan["sig"]
    if key not in _nc_cache:
        _nc_cache[key] = _build(plan)
    return _nc_cache[key]


# ---------------------------------------------------------------- kernel
def kernel(hand_verts, obj_verts, obj_split_sizes, exterior_hand, exterior_obj):
    import ml_dtypes

    hv = np.ascontiguousarray(hand_verts, dtype=np.float32) * CS
    ov = np.ascontiguousarray(obj_verts, dtype=np.float32) * CS
    splits = np.asarray(obj_split_sizes).astype(np.int64).reshape(B)
    eh = np.asarray(exterior_hand).astype(bool).reshape(B, NH)
    eo = np.asarray(exterior_obj).astype(bool).reshape(B, NO)
    valid = np.arange(NO)[None, :] < splits[:, None]
    interior = (~eo) & valid

    xx = (hv * hv).sum(-1).astype(np.float32)
    yy = (ov * ov).sum(-1).astype(np.float32)

    def split3(x):
        x0 = x.astype(ml_dtypes.bfloat16).astype(np.float32)
        r = x - x0
        x1 = r.astype(ml_dtypes.bfloat16).astype(np.float32)
        return x0, x1, r - x1

    o0, o1, o2 = split3(ov)
    h0, h1, h2 = split3(hv)
    y0, y1, y2 = split3(yy)
    x0, x1, x2 = split3(xx)
    xres = x1 + x2  # per-hand-vert residual, added post-min
    yres = y1 + y2  # per-obj-vert residual, added post-min
    A_SEQ = [o0, o0, o1, o1, o0, o2]
    B_SEQ = [h0, h1, h0, h1, h2, h0]

    # ---------- per-batch geometry plans ----------
    plans = []
    for b in range(B):
        v = int(splits[b])
        hvd = hv[b].astype(np.float64) / float(CS)
        ovd = ov[b, :v].astype(np.float64) / float(CS)
        hord = _kd_order(hvd, SC)
        oord = _kd_order(ovd, FB)
        hs = hvd[hord]
        os_ = ovd[oord]
        big_need = _needed_blocks(hs, os_, os_[::4])
        io_mask = interior[b, :v][oord]
        iidx = np.nonzero(io_mask)[0]
        ipts = os_[iidx]
        iord2 = _kd_order(ipts, SC)
        ipts = ipts[iord2]
        iglob = oord[iidx[iord2]]
        small_need = _needed_blocks(ipts, hs, hs[::2])
        nbk_big = [max(1, (len(n) + BPH - 1) // BPH) for n in big_need]
        nbk_small = [max(1, (len(n) + BPH - 1) // BPH) for n in small_need]
        plans.append({
            "v": v, "hord": hord, "oord": oord, "iglob": iglob,
            "big_need": big_need, "small_need": small_need,
            "nbk_big": nbk_big, "nbk_small": nbk_small,
            "load": sum(nbk_big) + sum(nbk_small),
        })

    # ---------- batch -> (core, slot) snake by load ----------
    order = np.argsort([-plans[b]["load"] for b in range(B)], kind="stable")
    slot_batches = np.empty((BPC, NCORES), np.int64)
    for s in range(BPC):
        seg = order[s * NCORES:(s + 1) * NCORES]
        if s % 2 == 1:
            seg = seg[::-1]
        slot_batches[s] = seg

    # ---------- unified rank-pure schedule ----------
    def unify(pass_key, nranks_per_slot):
        # Snake-deal sub-chunks (sorted desc by bank count) into the 4
        # classes so every class's rank-r count is as similar as possible.
        rank_maps = {}
        nbk = []
        for s in range(BPC):
            nr = nranks_per_slot[s]
            ranks = [1] * nr
            for core in range(NCORES):
                p = plans[slot_batches[s, core]]
                counts = p[pass_key]
                order_q = sorted(range(len(counts)), key=lambda q: -counts[q])
                percls = [[] for _ in range(4)]
                for r, q in enumerate(order_q):
                    c = r % 4 if (r // 4) % 2 == 0 else 3 - r % 4
                    percls[c].append(q)
                for c in range(4):
                    rank_maps[(core, s, c)] = percls[c]
                    for r, q in enumerate(percls[c]):
                        if r < nr:
                            ranks[r] = max(ranks[r], counts[q])
            nbk.append(ranks)
        bank_rank = []
        ranges = []
        slot_rank_base = []
        rglob = 0
        for s in range(BPC):
            slot_rank_base.append(rglob)
            for r, nb in enumerate(nbk[s]):
                lo = len(bank_rank)
                bank_rank.extend([rglob] * nb)
                ranges.append((lo, lo + nb))
                rglob += 1
        return (rank_maps, nbk, bank_rank, ranges, len(bank_rank), rglob,
                slot_rank_base)

    nranks_b = [max(len(range(c, NSC, 4)) for c in range(4))
                for s in range(BPC)]  # 7
    (rmap_b, nbk_b, bank_rank_b, ranges_b, TOTBb, Rb, srb_b) = \
        unify("nbk_big", nranks_b)

    nranks_s = []
    for s in range(BPC):
        mx = 1
        for core in range(NCORES):
            p = plans[slot_batches[s, core]]
            for c in range(4):
                mx = max(mx, len(range(c, len(p["nbk_small"]), 4)))
        nranks_s.append(mx)
    (rmap_s, nbk_s, bank_rank_s, ranges_s, TOTBs, Rs, srb_s) = \
        unify("nbk_small", nranks_s)

    plan = {
        "totb_big": TOTBb, "totb_small": TOTBs,
        "r_big": Rb, "r_small": Rs,
        "bank_rank_big": bank_rank_b, "bank_rank_small": bank_rank_s,
        "ranges_big": ranges_b, "ranges_small": ranges_s,
    }
    plan["sig"] = repr((TOTBb, TOTBs, Rb, Rs, bank_rank_b, bank_rank_s))

    # ---------- per-core packing ----------
    in_maps = []
    for core in range(NCORES):
        rb = np.zeros((4, KD, TOTBb * HB), np.float32)
        wb = np.zeros((4, KD, Rb * SC), np.float32)
        rs = np.zeros((4, KD, TOTBs * HB), np.float32)
        ws = np.zeros((4, KD, Rs * SC), np.float32)
        me = np.zeros((128, Rb), np.float32)
        mi = np.zeros((128, Rb), np.float32)
        mo = np.zeros((128, Rs), np.float32)
        xr = np.zeros((128, Rb), np.float32)
        yr = np.zeros((128, Rs), np.float32)
        for s in range(BPC):
            b = int(slot_batches[s, core])
            p = plans[b]
            v = p["v"]
            hord, oord = p["hord"], p["oord"]
            nop = ((v + FB - 1) // FB) * FB
            # big pass: rhs = obj cols, lhsT = hand lanes
            ocols = np.zeros((KD, nop), np.float32)
            for j in range(6):
                for d in range(3):
                    ocols[3 * j + d, :v] = A_SEQ[j][b, oord, d]
            ocols[18, :v] = y0[b, oord]
            ocols[18, v:] = BIG
            ocols[19, :v] = y1[b, oord]
            ocols[20, :v] = y2[b, oord]
            ocols[21, :v] = 1.0
            hlanes = np.zeros((KD, NHP), np.float32)
            for j in range(6):
                for d in range(3):
                    hlanes[3 * j + d, :NH] = -2.0 * B_SEQ[j][b, hord, d]
            hlanes[18:21, :NH] = 1.0
            hlanes[21, :NH] = x0[b, hord]
            ehb = eh[b][hord]
            xresb = xres[b, hord]

            # small pass: rhs = hand cols, lhsT = interior obj lanes
            hcols = np.zeros((KD, NHP), np.float32)
            for j in range(6):
                for d in range(3):
                    hcols[3 * j + d, :NH] = -2.0 * B_SEQ[j][b, hord, d]
            hcols[18, :NH] = x0[b, hord]
            hcols[18, NH:] = PAD
            hcols[19, :NH] = x1[b, hord]
            hcols[20, :NH] = x2[b, hord]
            hcols[21, :NH] = 1.0
            iglob = p["iglob"]
            ni = len(iglob)
            olanes = np.zeros((KD, max(ni, 1)), np.float32)
            for j in range(6):
                for d in range(3):
                    olanes[3 * j + d, :ni] = A_SEQ[j][b, iglob, d]
            olanes[18:21, :ni] = 1.0
            olanes[21, :ni] = y0[b, iglob]
            yresb = yres[b, iglob]

            def pack(rank_maps, nbk_slot, rbase, ranges_of, need_key, cols,
                     lanes, nlanes_of, rarr, warr, mask_sinks):
                for c in range(4):
                    subs = rank_maps[(core, s, c)]
                    for r in range(len(nbk_slot)):
                        rg = rbase + r
                        nbu = nbk_slot[r]
                        lo = ranges_of[rg][0]
                        if r < len(subs):
                            q = subs[r]
                            nl = nlanes_of(q)
                            warr[c, :, rg * SC:rg * SC + nl] = \
                                lanes[:, q * SC:q * SC + nl]
                            blocks = p[need_key][q]
                            nblk = len(blocks)
                            tot = nbu * BPH
                            gath = [blocks[j % nblk] for j in range(tot)]
                            gcols = np.concatenate(
                                [np.arange(g * FB, (g + 1) * FB)
                                 for g in gath])
                            rarr[c, :, lo * HB:lo * HB + tot * FB] = \
                                cols[:, gcols]
                            for msk, vals in mask_sinks(q, nl):
                                msk[32 * c:32 * c + nl, rg] = vals
                        else:
                            rarr[c, :, lo * HB:(lo + nbu) * HB] = \
                                np.tile(cols[:, 0:HB], (1, nbu))

            pack(rmap_b, nbk_b[s], srb_b[s], ranges_b, "big_need", ocols,
                 hlanes, lambda q: min(SC, NH - q * SC), rb, wb,
                 lambda q, nl: [(me, ehb[q * SC:q * SC + nl]),
                                (mi, ~ehb[q * SC:q * SC + nl]),
                                (xr, xresb[q * SC:q * SC + nl])])
            pack(rmap_s, nbk_s[s], srb_s[s], ranges_s, "small_need", hcols,
                 olanes, lambda q: min(SC, ni - q * SC), rs, ws,
                 lambda q, nl: [(mo, 1.0),
                                (yr, yresb[q * SC:q * SC + nl])])

        in_maps.append({
            "rb": rb.astype(ml_dtypes.bfloat16),
            "wb": wb.astype(ml_dtypes.bfloat16),
            "rs": rs.astype(ml_dtypes.bfloat16),
            "ws": ws.astype(ml_dtypes.bfloat16),
            "mask_e": me, "mask_i": mi, "mask_o": mo,
            "xres": xr, "yres": yr,
        })

    nc = _get_nc(plan)
    res = run_bass_kernel_spmd(nc, in_maps, list(range(NCORES))).results

    nums = np.zeros(3, np.float64)
    for rr in res:
        nums += rr["out"][:, 0:3].astype(np.float64).sum(axis=0)
    dens = np.array([eh.sum(), (~eh).sum(), interior.sum()], dtype=np.float64)
    out = np.where(dens > 0, 0.025 * nums / np.maximum(dens, 1.0), 0.0)
    return out.astype(np.float32)


# revision 29
# speedup vs baseline: 1.1613x; 1.0089x over previous
"""ContactLoss Trainium2 kernel v3.2 (8 NeuronCores, batch data-parallel,
spatially-pruned KNN).

Big pass (minho): hand verts kd-sorted into sub-chunks of 32; valid obj
verts kd-sorted into 32-col blocks. Host computes exact lower bounds
(point-to-block-bbox) and upper bounds (dist to obj reps) and keeps, per
sub-chunk, only the obj blocks that can contain a nearest neighbour.
Needed blocks are host-gathered into dense 256-col half-banks. On
device, 16-way PE tiling (32x32 diagonal tiles, tile_position=
(32c,32c)) computes 4 sub-chunks (one per 32-lane class) concurrently.
Schedule is rank-pure: rank r = the r-th-largest sub-chunk of each
class (sorted by half-bank count); nbk[slot][r] = max count over
classes and cores, so a single SPMD program serves all 8 cores. Wave =
4 consecutive halves (2 PSUM banks, 4-deep pipeline); one
[22,32]x[22,256] matmul per (half, class); consume reduces the wave to
per-half minima columns of MP; one tiny contiguous reduce per rank
collapses its halves into MH, then masked sums produce the three loss
numerators. Small pass (minoh for interior obj): same machinery, roles
swapped. Classes pad short ranks by repeating their own blocks
(min-safe).

d2 precision: xx+yy-2xy with bf16 split products (6 pairs). The lhsT
side carries only the leading bf16 term of its squared-norm (x0 big /
y0 small); the residual (x1+x2 or y1+y2) is constant per output lane,
so it is added after the min on-device (keeps the f16 drain centred
near zero). COORD_SCALE keeps f16 drains in range.
"""

import sys
from contextlib import ExitStack

import numpy as np

sys.path.insert(0, "/opt/trn_rl_repo")

import concourse.mybir as mybir  # noqa: E402
import concourse.tile as tile  # noqa: E402
from concourse import bacc  # noqa: E402
from concourse.bass_utils import run_bass_kernel_spmd  # noqa: E402

B, NH, NO = 32, 778, 8192
NCORES = 8
BPC = B // NCORES
SC = 32  # sub-chunk lanes
NSC = (NH + SC - 1) // SC  # 25 hand sub-chunks
FB = 16  # fine block cols for the need test / gather granularity
BK = 512  # PSUM bank columns
HB = 256  # half-bank columns (scheduling granularity)
BPH = HB // FB  # fine blocks per half-bank
KD = 22
CS = np.float32(16.0)
BIG = np.float32(49152.0)
PAD = np.float32(8192.0)
NHP = ((NH + FB - 1) // FB) * FB  # 800

F32 = mybir.dt.float32
F16 = mybir.dt.float16
BF16 = mybir.dt.bfloat16
MIN = mybir.AluOpType.min
MULT = mybir.AluOpType.mult
ADD = mybir.AluOpType.add
AX = mybir.AxisListType.X
AF = mybir.ActivationFunctionType

_nc_cache = {}


# ---------------------------------------------------------------- geometry
def _kd_order(pts, leaf):
    """Recursive median split on widest axis -> contiguous leaves."""
    out = []

    def rec(ids):
        if len(ids) <= leaf:
            out.append(ids)
            return
        p = pts[ids]
        ax = int(np.argmax(p.max(0) - p.min(0)))
        k = len(ids) // 2
        part = np.argpartition(p[:, ax], k)
        rec(ids[part[:k]])
        rec(ids[part[k:]])

    sys.setrecursionlimit(100000)
    rec(np.arange(len(pts)))
    return np.concatenate(out)


def _box_dist2(q, blo, bhi):
    d = np.maximum(np.maximum(blo[None, :, :] - q[:, None, :],
                              q[:, None, :] - bhi[None, :, :]), 0.0)
    return (d * d).sum(-1)


def _needed_blocks(lanes_pts, sorted_pts, reps):
    """Per sub-chunk-of-32 of lanes_pts: indices of needed FB-blocks."""
    n = len(sorted_pts)
    starts = np.arange(0, n, FB)
    blo = np.minimum.reduceat(sorted_pts, starts)
    bhi = np.maximum.reduceat(sorted_pts, starts)
    d2 = ((lanes_pts[:, None, :] - reps[None, :, :]) ** 2).sum(-1)
    u2 = d2.min(1)
    lb2 = _box_dist2(lanes_pts, blo, bhi)
    need = lb2 <= u2[:, None] * (1.0 + 1e-9) + 1e-12
    nsub = (len(lanes_pts) + SC - 1) // SC
    return [np.nonzero(need[s * SC:(s + 1) * SC].any(0))[0]
            for s in range(nsub)]


# ---------------------------------------------------------------- device
def _build(plan):
    nc = bacc.Bacc("TRN2", target_bir_lowering=False, debug=False,
                   num_devices=NCORES)
    TOTBb, TOTBs = plan["totb_big"], plan["totb_small"]
    Rb, Rs = plan["r_big"], plan["r_small"]

    rb_d = nc.declare_dram_parameter("rb", [4, KD, TOTBb * HB], BF16,
                                     isOutput=False)
    wb_d = nc.declare_dram_parameter("wb", [4, KD, Rb * SC], BF16,
                                     isOutput=False)
    rs_d = nc.declare_dram_parameter("rs", [4, KD, TOTBs * HB], BF16,
                                     isOutput=False)
    ws_d = nc.declare_dram_parameter("ws", [4, KD, Rs * SC], BF16,
                                     isOutput=False)
    me_d = nc.declare_dram_parameter("mask_e", [128, Rb], F32, isOutput=False)
    mi_d = nc.declare_dram_parameter("mask_i", [128, Rb], F32, isOutput=False)
    mo_d = nc.declare_dram_parameter("mask_o", [128, Rs], F32, isOutput=False)
    xr_d = nc.declare_dram_parameter("xres", [128, Rb], F32, isOutput=False)
    yr_d = nc.declare_dram_parameter("yres", [128, Rs], F32, isOutput=False)
    out_d = nc.declare_dram_parameter("out", [128, 4], F32, isOutput=True)

    with ExitStack() as ctx:
        tc = ctx.enter_context(tile.TileContext(nc))
        singles = ctx.enter_context(tc.tile_pool(name="singles", bufs=1))
        d16p = ctx.enter_context(tc.tile_pool(name="d16p", bufs=4))
        l1p = ctx.enter_context(tc.tile_pool(name="l1p", bufs=2))
        l2p = ctx.enter_context(tc.tile_pool(name="l2p", bufs=2))
        l3p = ctx.enter_context(tc.tile_pool(name="l3p", bufs=2))

        RB = singles.tile([128, TOTBb, HB], BF16)
        WB = singles.tile([128, Rb, SC], BF16)
        RS = singles.tile([128, TOTBs, HB], BF16)
        WS = singles.tile([128, Rs, SC], BF16)
        RBf = RB.rearrange("p a b -> p (a b)")
        WBf = WB.rearrange("p a b -> p (a b)")
        RSf = RS.rearrange("p a b -> p (a b)")
        WSf = WS.rearrange("p a b -> p (a b)")
        # scalar queue must stay free for drains: bulk DMA on sync+gpsimd
        dq = [nc.sync, nc.gpsimd]
        for c in range(4):
            dq[c % 2].dma_start(out=WBf[32 * c:32 * c + KD, :], in_=wb_d[c])
            dq[(c + 1) % 2].dma_start(
                out=WSf[32 * c:32 * c + KD, :], in_=ws_d[c])
        # rhs cells, chunked; chunk0 of all classes first for fast start.
        # sync is HW-DGE (fast); gpsimd is SW-DGE (slow drains) - use 3:1.
        qi = [0]

        def rchunks(totb, RF, r_d, queues, early=0):
            chunks = [(0, min(8, totb))]
            lo = chunks[0][1]
            while lo < totb:
                hi = min(lo + 24, totb)
                chunks.append((lo, hi))
                lo = hi
            for k, (lo, hi) in enumerate(chunks):
                for c in range(4):
                    if k < early and c >= 2:
                        q = nc.scalar  # idle until first drain
                    else:
                        q = queues[qi[0] % len(queues)]
                        qi[0] += 1
                    q.dma_start(
                        out=RF[32 * c:32 * c + KD, lo * HB:hi * HB],
                        in_=r_d[c, :, lo * HB:hi * HB])

        rchunks(TOTBb, RBf, rb_d, [nc.sync, nc.gpsimd], early=2)
        rchunks(TOTBs, RSf, rs_d, [nc.scalar, nc.sync, nc.gpsimd, nc.sync])
        # prewarm ACT spline tables (sqrt/tanh) while DMAs land
        warm = singles.tile([128, 1], F32)
        nc.vector.memset(warm, 1.0)
        nc.scalar.sqrt(warm, warm)
        nc.scalar.activation(warm, warm, AF.Tanh, scale=1.0)

        me = singles.tile([128, Rb], F32)
        nc.scalar.dma_start(out=me, in_=me_d[:, :])
        mi = singles.tile([128, Rb], F32)
        nc.scalar.dma_start(out=mi, in_=mi_d[:, :])
        mo = singles.tile([128, Rs], F32)
        nc.scalar.dma_start(out=mo, in_=mo_d[:, :])
        xr = singles.tile([128, Rb], F32)
        nc.scalar.dma_start(out=xr, in_=xr_d[:, :])
        yr = singles.tile([128, Rs], F32)
        nc.scalar.dma_start(out=yr, in_=yr_d[:, :])

        MPB = singles.tile([128, TOTBb + 8], F32)  # per-half partial minima
        MPS = singles.tile([128, TOTBs + 8], F32)
        MHB = singles.tile([128, Rb], F32)  # per-rank minima
        MHS = singles.tile([128, Rs], F32)

        act_t = [0.0]
        dve_t = [0.0]

        def consume(ps, H, NHW, MP, h0):
            # dst: MP[:, h0:h0+NHW] per-half minima (+1 pad col if odd)
            nout = 2 * H
            dst = MP[:, h0:h0 + nout]
            aA = (512 * H + 352) / 1.2
            dA = (232 + 288 * H) / 0.96
            dC = (120 + 512 * H) / 0.96
            costA = max(act_t[0] + aA, dve_t[0] + dA)
            costC = max(act_t[0], dve_t[0] + dC)
            if costC < costA:
                psv = ps[:, 0:H, :].rearrange("p h (t x) -> p (h t) x", t=2)
                nc.vector.tensor_reduce(dst, psv, axis=AX, op=MIN)
                dve_t[0] += dC
                return
            d16 = d16p.tile([128, 4, BK], F16)
            nc.scalar.copy(d16[:, 0:H, :], ps[:, 0:H, :])
            l1 = l1p.tile([128, 4, 2, BK // 4], F16)
            d16v = d16.rearrange("p h (t x) -> p h t x", t=2)
            nc.vector.tensor_tensor(
                l1[:, 0:H, :, :], d16v[:, 0:H, :, 0:BK // 4],
                d16v[:, 0:H, :, BK // 4:BK // 2], MIN)
            l2 = l2p.tile([128, 4, 2, BK // 8], F16)
            nc.vector.tensor_tensor(
                l2[:, 0:H, :, :], l1[:, 0:H, :, 0:BK // 8],
                l1[:, 0:H, :, BK // 8:BK // 4], MIN)
            l3 = l3p.tile([128, 4, 2, BK // 16], F16)
            nc.vector.tensor_tensor(
                l3[:, 0:H, :, :], l2[:, 0:H, :, 0:BK // 16],
                l2[:, 0:H, :, BK // 16:BK // 8], MIN)
            l3v = l3.rearrange("p h t x -> p (h t) x")
            nc.vector.tensor_reduce(dst, l3v[:, 0:nout, :], axis=AX, op=MIN)
            act_t[0] += aA
            dve_t[0] += dA

        def run_pass(TOTH, half_rank, RT, WT, MP, MH, ranges):
            # halves of 256 cols; wave = up to 8 halves = 4 PSUM banks
            nw = (TOTH + 3) // 4
            with tc.tile_pool(name="ps", bufs=4, space="PSUM") as psp:
                for w in range(nw):
                    h0 = 4 * w
                    NHW = min(4, TOTH - h0)
                    H = (NHW + 1) // 2
                    ps = psp.tile([128, 2, BK], F32)
                    for c in range(4):
                        for j in range(NHW):
                            r = half_rank[h0 + j]
                            nc.tensor.matmul(
                                ps[32 * c:32 * c + 32, j // 2,
                                   (j % 2) * HB:(j % 2) * HB + HB],
                                WT[32 * c:32 * c + KD, r * SC:(r + 1) * SC],
                                RT[32 * c:32 * c + KD,
                                   (h0 + j) * HB:(h0 + j + 1) * HB],
                                start=True, stop=True,
                                tile_position=(32 * c, 32 * c),
                            )
                        if NHW % 2 == 1:
                            # fill odd tail with a repeat (keeps reduce valid)
                            r = half_rank[h0 + NHW - 1]
                            nc.tensor.matmul(
                                ps[32 * c:32 * c + 32, NHW // 2, HB:2 * HB],
                                WT[32 * c:32 * c + KD, r * SC:(r + 1) * SC],
                                RT[32 * c:32 * c + KD,
                                   (h0 + NHW - 1) * HB:(h0 + NHW) * HB],
                                start=True, stop=True,
                                tile_position=(32 * c, 32 * c),
                            )
                    consume(ps, H, NHW, MP, h0)
                    # emit rank reduces whose halves completed in this wave
                    for r, (lo, hi) in enumerate(ranges):
                        if h0 < hi <= h0 + NHW:
                            nc.vector.tensor_reduce(
                                MH[:, r:r + 1], MP[:, lo:hi], axis=AX, op=MIN)

        outsb = singles.tile([128, 4], F32)
        jh = singles.tile([128, Rb], F32)
        jh2 = singles.tile([128, Rb], F32)
        jo = singles.tile([128, Rs], F32)
        run_pass(TOTBb, plan["bank_rank_big"], RBf, WBf, MPB, MHB,
                 plan["ranges_big"])
        run_pass(TOTBs, plan["bank_rank_small"], RSf, WSf, MPS, MHS,
                 plan["ranges_small"])
        # end phase, batched by ACT table set
        nc.vector.tensor_tensor(MHB, MHB, xr, ADD)
        nc.vector.tensor_tensor(MHS, MHS, yr, ADD)
        for MH in (MHB, MHS):
            nc.vector.tensor_scalar_max(MH, MH, 0.0)
            nc.vector.tensor_scalar_min(MH, MH, 1.0e4)
        nc.scalar.sqrt(MHB, MHB)
        nc.scalar.sqrt(MHS, MHS)
        nc.scalar.activation(MHB, MHB, AF.Tanh, scale=1.0 / (0.025 * float(CS)))
        nc.scalar.activation(MHS, MHS, AF.Tanh, scale=1.0 / (0.025 * float(CS)))
        nc.vector.tensor_tensor(jh, MHB, me, MULT)
        nc.vector.tensor_reduce(outsb[:, 0:1], jh, axis=AX, op=ADD)
        nc.vector.tensor_tensor(jh2, MHB, mi, MULT)
        nc.vector.tensor_reduce(outsb[:, 1:2], jh2, axis=AX, op=ADD)
        nc.vector.tensor_tensor(jo, MHS, mo, MULT)
        nc.vector.tensor_reduce(outsb[:, 2:3], jo, axis=AX, op=ADD)
        nc.vector.memset(outsb[:, 3:4], 0.0)
        nc.sync.dma_start(out=out_d[:, :], in_=outsb)
    nc.compile()
    return nc


def _get_nc(plan):
    key = plan["sig"]
    if key not in _nc_cache:
        _nc_cache[key] = _build(plan)
    return _nc_cache[key]


# ---------------------------------------------------------------- kernel
def kernel(hand_verts, obj_verts, obj_split_sizes, exterior_hand, exterior_obj):
    import ml_dtypes

    hv = np.ascontiguousarray(hand_verts, dtype=np.float32) * CS
    ov = np.ascontiguousarray(obj_verts, dtype=np.float32) * CS
    splits = np.asarray(obj_split_sizes).astype(np.int64).reshape(B)
    eh = np.asarray(exterior_hand).astype(bool).reshape(B, NH)
    eo = np.asarray(exterior_obj).astype(bool).reshape(B, NO)
    valid = np.arange(NO)[None, :] < splits[:, None]
    interior = (~eo) & valid

    xx = (hv * hv).sum(-1).astype(np.float32)
    yy = (ov * ov).sum(-1).astype(np.float32)

    def split3(x):
        x0 = x.astype(ml_dtypes.bfloat16).astype(np.float32)
        r = x - x0
        x1 = r.astype(ml_dtypes.bfloat16).astype(np.float32)
        return x0, x1, r - x1

    o0, o1, o2 = split3(ov)
    h0, h1, h2 = split3(hv)
    y0, y1, y2 = split3(yy)
    x0, x1, x2 = split3(xx)
    xres = x1 + x2  # per-hand-vert residual, added post-min
    yres = y1 + y2  # per-obj-vert residual, added post-min
    A_SEQ = [o0, o0, o1, o1, o0, o2]
    B_SEQ = [h0, h1, h0, h1, h2, h0]

    # ---------- per-batch geometry plans ----------
    plans = []
    for b in range(B):
        v = int(splits[b])
        hvd = hv[b].astype(np.float64) / float(CS)
        ovd = ov[b, :v].astype(np.float64) / float(CS)
        hord = _kd_order(hvd, SC)
        oord = _kd_order(ovd, FB)
        hs = hvd[hord]
        os_ = ovd[oord]
        big_need = _needed_blocks(hs, os_, os_[::4])
        io_mask = interior[b, :v][oord]
        iidx = np.nonzero(io_mask)[0]
        ipts = os_[iidx]
        iord2 = _kd_order(ipts, SC)
        ipts = ipts[iord2]
        iglob = oord[iidx[iord2]]
        small_need = _needed_blocks(ipts, hs, hs[::2])
        nbk_big = [max(1, (len(n) + BPH - 1) // BPH) for n in big_need]
        nbk_small = [max(1, (len(n) + BPH - 1) // BPH) for n in small_need]
        plans.append({
            "v": v, "hord": hord, "oord": oord, "iglob": iglob,
            "big_need": big_need, "small_need": small_need,
            "nbk_big": nbk_big, "nbk_small": nbk_small,
            "load": sum(nbk_big) + sum(nbk_small),
        })

    # ---------- batch -> (core, slot) snake by load ----------
    order = np.argsort([-plans[b]["load"] for b in range(B)], kind="stable")
    slot_batches = np.empty((BPC, NCORES), np.int64)
    for s in range(BPC):
        seg = order[s * NCORES:(s + 1) * NCORES]
        if s % 2 == 1:
            seg = seg[::-1]
        slot_batches[s] = seg

    # ---------- unified rank-pure schedule ----------
    def unify(pass_key, nranks_per_slot):
        # Snake-deal sub-chunks (sorted desc by bank count) into the 4
        # classes so every class's rank-r count is as similar as possible.
        rank_maps = {}
        nbk = []
        for s in range(BPC):
            nr = nranks_per_slot[s]
            ranks = [1] * nr
            for core in range(NCORES):
                p = plans[slot_batches[s, core]]
                counts = p[pass_key]
                order_q = sorted(range(len(counts)), key=lambda q: -counts[q])
                percls = [[] for _ in range(4)]
                for r, q in enumerate(order_q):
                    c = r % 4 if (r // 4) % 2 == 0 else 3 - r % 4
                    percls[c].append(q)
                for c in range(4):
                    rank_maps[(core, s, c)] = percls[c]
                    for r, q in enumerate(percls[c]):
                        if r < nr:
                            ranks[r] = max(ranks[r], counts[q])
            nbk.append(ranks)
        bank_rank = []
        ranges = []
        slot_rank_base = []
        rglob = 0
        for s in range(BPC):
            slot_rank_base.append(rglob)
            for r, nb in enumerate(nbk[s]):
                lo = len(bank_rank)
                bank_rank.extend([rglob] * nb)
                ranges.append((lo, lo + nb))
                rglob += 1
        return (rank_maps, nbk, bank_rank, ranges, len(bank_rank), rglob,
                slot_rank_base)

    nranks_b = [max(len(range(c, NSC, 4)) for c in range(4))
                for s in range(BPC)]  # 7
    (rmap_b, nbk_b, bank_rank_b, ranges_b, TOTBb, Rb, srb_b) = \
        unify("nbk_big", nranks_b)

    nranks_s = []
    for s in range(BPC):
        mx = 1
        for core in range(NCORES):
            p = plans[slot_batches[s, core]]
            for c in range(4):
                mx = max(mx, len(range(c, len(p["nbk_small"]), 4)))
        nranks_s.append(mx)
    (rmap_s, nbk_s, bank_rank_s, ranges_s, TOTBs, Rs, srb_s) = \
        unify("nbk_small", nranks_s)

    plan = {
        "totb_big": TOTBb, "totb_small": TOTBs,
        "r_big": Rb, "r_small": Rs,
        "bank_rank_big": bank_rank_b, "bank_rank_small": bank_rank_s,
        "ranges_big": ranges_b, "ranges_small": ranges_s,
    }
    plan["sig"] = repr((TOTBb, TOTBs, Rb, Rs, bank_rank_b, bank_rank_s))

    # ---------- per-core packing ----------
    in_maps = []
    for core in range(NCORES):
        rb = np.zeros((4, KD, TOTBb * HB), np.float32)
        wb = np.zeros((4, KD, Rb * SC), np.float32)
        rs = np.zeros((4, KD, TOTBs * HB), np.float32)
        ws = np.zeros((4, KD, Rs * SC), np.float32)
        me = np.zeros((128, Rb), np.float32)
        mi = np.zeros((128, Rb), np.float32)
        mo = np.zeros((128, Rs), np.float32)
        xr = np.zeros((128, Rb), np.float32)
        yr = np.zeros((128, Rs), np.float32)
        for s in range(BPC):
            b = int(slot_batches[s, core])
            p = plans[b]
            v = p["v"]
            hord, oord = p["hord"], p["oord"]
            nop = ((v + FB - 1) // FB) * FB
            # big pass: rhs = obj cols, lhsT = hand lanes
            ocols = np.zeros((KD, nop), np.float32)
            for j in range(6):
                for d in range(3):
                    ocols[3 * j + d, :v] = A_SEQ[j][b, oord, d]
            ocols[18, :v] = y0[b, oord]
            ocols[18, v:] = BIG
            ocols[19, :v] = y1[b, oord]
            ocols[20, :v] = y2[b, oord]
            ocols[21, :v] = 1.0
            hlanes = np.zeros((KD, NHP), np.float32)
            for j in range(6):
                for d in range(3):
                    hlanes[3 * j + d, :NH] = -2.0 * B_SEQ[j][b, hord, d]
            hlanes[18:21, :NH] = 1.0
            hlanes[21, :NH] = x0[b, hord]
            ehb = eh[b][hord]
            xresb = xres[b, hord]

            # small pass: rhs = hand cols, lhsT = interior obj lanes
            hcols = np.zeros((KD, NHP), np.float32)
            for j in range(6):
                for d in range(3):
                    hcols[3 * j + d, :NH] = -2.0 * B_SEQ[j][b, hord, d]
            hcols[18, :NH] = x0[b, hord]
            hcols[18, NH:] = PAD
            hcols[19, :NH] = x1[b, hord]
            hcols[20, :NH] = x2[b, hord]
            hcols[21, :NH] = 1.0
            iglob = p["iglob"]
            ni = len(iglob)
            olanes = np.zeros((KD, max(ni, 1)), np.float32)
            for j in range(6):
                for d in range(3):
                    olanes[3 * j + d, :ni] = A_SEQ[j][b, iglob, d]
            olanes[18:21, :ni] = 1.0
            olanes[21, :ni] = y0[b, iglob]
            yresb = yres[b, iglob]

            def pack(rank_maps, nbk_slot, rbase, ranges_of, need_key, cols,
                     lanes, nlanes_of, rarr, warr, mask_sinks):
                for c in range(4):
                    subs = rank_maps[(core, s, c)]
                    for r in range(len(nbk_slot)):
                        rg = rbase + r
                        nbu = nbk_slot[r]
                        lo = ranges_of[rg][0]
                        if r < len(subs):
                            q = subs[r]
                            nl = nlanes_of(q)
                            warr[c, :, rg * SC:rg * SC + nl] = \
                                lanes[:, q * SC:q * SC + nl]
                            blocks = p[need_key][q]
                            nblk = len(blocks)
                            tot = nbu * BPH
                            gath = [blocks[j % nblk] for j in range(tot)]
                            gcols = np.concatenate(
                                [np.arange(g * FB, (g + 1) * FB)
                                 for g in gath])
                            rarr[c, :, lo * HB:lo * HB + tot * FB] = \
                                cols[:, gcols]
                            for msk, vals in mask_sinks(q, nl):
                                msk[32 * c:32 * c + nl, rg] = vals
                        else:
                            rarr[c, :, lo * HB:(lo + nbu) * HB] = \
                                np.tile(cols[:, 0:HB], (1, nbu))

            pack(rmap_b, nbk_b[s], srb_b[s], ranges_b, "big_need", ocols,
                 hlanes, lambda q: min(SC, NH - q * SC), rb, wb,
                 lambda q, nl: [(me, ehb[q * SC:q * SC + nl]),
                                (mi, ~ehb[q * SC:q * SC + nl]),
                                (xr, xresb[q * SC:q * SC + nl])])
            pack(rmap_s, nbk_s[s], srb_s[s], ranges_s, "small_need", hcols,
                 olanes, lambda q: min(SC, ni - q * SC), rs, ws,
                 lambda q, nl: [(mo, 1.0),
                                (yr, yresb[q * SC:q * SC + nl])])

        in_maps.append({
            "rb": rb.astype(ml_dtypes.bfloat16),
            "wb": wb.astype(ml_dtypes.bfloat16),
            "rs": rs.astype(ml_dtypes.bfloat16),
            "ws": ws.astype(ml_dtypes.bfloat16),
            "mask_e": me, "mask_i": mi, "mask_o": mo,
            "xres": xr, "yres": yr,
        })

    nc = _get_nc(plan)
    res = run_bass_kernel_spmd(nc, in_maps, list(range(NCORES))).results

    nums = np.zeros(3, np.float64)
    for rr in res:
        nums += rr["out"][:, 0:3].astype(np.float64).sum(axis=0)
    dens = np.array([eh.sum(), (~eh).sum(), interior.sum()], dtype=np.float64)
    out = np.where(dens > 0, 0.025 * nums / np.maximum(dens, 1.0), 0.0)
    return out.astype(np.float32)


# revision 30
# speedup vs baseline: 1.2199x; 1.0505x over previous
"""ContactLoss Trainium2 kernel v3.2 (8 NeuronCores, batch data-parallel,
spatially-pruned KNN).

Big pass (minho): hand verts kd-sorted into sub-chunks of 32; valid obj
verts kd-sorted into 32-col blocks. Host computes exact lower bounds
(point-to-block-bbox) and upper bounds (dist to obj reps) and keeps, per
sub-chunk, only the obj blocks that can contain a nearest neighbour.
Needed blocks are host-gathered into dense 256-col half-banks. On
device, 16-way PE tiling (32x32 diagonal tiles, tile_position=
(32c,32c)) computes 4 sub-chunks (one per 32-lane class) concurrently.
Schedule is rank-pure: rank r = the r-th-largest sub-chunk of each
class (sorted by half-bank count); nbk[slot][r] = max count over
classes and cores, so a single SPMD program serves all 8 cores. Wave =
4 consecutive halves (2 PSUM banks, 4-deep pipeline); one
[22,32]x[22,256] matmul per (half, class); consume reduces the wave to
per-half minima columns of MP; one tiny contiguous reduce per rank
collapses its halves into MH, then masked sums produce the three loss
numerators. Small pass (minoh for interior obj): same machinery, roles
swapped. Classes pad short ranks by repeating their own blocks
(min-safe).

d2 precision: xx+yy-2xy with bf16 split products (6 pairs). The lhsT
side carries only the leading bf16 term of its squared-norm (x0 big /
y0 small); the residual (x1+x2 or y1+y2) is constant per output lane,
so it is added after the min on-device (keeps the f16 drain centred
near zero). COORD_SCALE keeps f16 drains in range.
"""

import sys
from contextlib import ExitStack

import numpy as np

sys.path.insert(0, "/opt/trn_rl_repo")

import concourse.mybir as mybir  # noqa: E402
import concourse.tile as tile  # noqa: E402
from concourse import bacc  # noqa: E402
from concourse.bass_utils import run_bass_kernel_spmd  # noqa: E402

B, NH, NO = 32, 778, 8192
NCORES = 8
BPC = B // NCORES
SC = 32  # sub-chunk lanes
NSC = (NH + SC - 1) // SC  # 25 hand sub-chunks
FB = 16  # fine block cols for the need test / gather granularity
BK = 512  # PSUM bank columns
HB = 256  # half-bank columns (scheduling granularity)
BPH = HB // FB  # fine blocks per half-bank
KD = 22
CS = np.float32(16.0)
BIG = np.float32(49152.0)
PAD = np.float32(8192.0)
NHP = ((NH + FB - 1) // FB) * FB  # 800

F32 = mybir.dt.float32
F16 = mybir.dt.float16
BF16 = mybir.dt.bfloat16
MIN = mybir.AluOpType.min
MULT = mybir.AluOpType.mult
ADD = mybir.AluOpType.add
AX = mybir.AxisListType.X
AF = mybir.ActivationFunctionType

_nc_cache = {}


# ---------------------------------------------------------------- geometry
def _kd_order(pts, leaf):
    """Recursive median split on widest axis -> contiguous leaves."""
    out = []

    def rec(ids):
        if len(ids) <= leaf:
            out.append(ids)
            return
        p = pts[ids]
        ax = int(np.argmax(p.max(0) - p.min(0)))
        k = len(ids) // 2
        part = np.argpartition(p[:, ax], k)
        rec(ids[part[:k]])
        rec(ids[part[k:]])

    sys.setrecursionlimit(100000)
    rec(np.arange(len(pts)))
    return np.concatenate(out)


def _box_dist2(q, blo, bhi):
    d = np.maximum(np.maximum(blo[None, :, :] - q[:, None, :],
                              q[:, None, :] - bhi[None, :, :]), 0.0)
    return (d * d).sum(-1)


def _needed_blocks(lanes_pts, sorted_pts, reps):
    """Per sub-chunk-of-32 of lanes_pts: indices of needed FB-blocks."""
    n = len(sorted_pts)
    starts = np.arange(0, n, FB)
    blo = np.minimum.reduceat(sorted_pts, starts)
    bhi = np.maximum.reduceat(sorted_pts, starts)
    d2 = ((lanes_pts[:, None, :] - reps[None, :, :]) ** 2).sum(-1)
    u2 = d2.min(1)
    lb2 = _box_dist2(lanes_pts, blo, bhi)
    need = lb2 <= u2[:, None] * (1.0 + 1e-9) + 1e-12
    nsub = (len(lanes_pts) + SC - 1) // SC
    return [np.nonzero(need[s * SC:(s + 1) * SC].any(0))[0]
            for s in range(nsub)]


# ---------------------------------------------------------------- device
def _build(plan):
    nc = bacc.Bacc("TRN2", target_bir_lowering=False, debug=False,
                   num_devices=NCORES)
    TOTBb, TOTBs = plan["totb_big"], plan["totb_small"]
    Rb, Rs = plan["r_big"], plan["r_small"]

    rb_d = nc.declare_dram_parameter("rb", [4, KD, TOTBb * HB], BF16,
                                     isOutput=False)
    wb_d = nc.declare_dram_parameter("wb", [4, KD, Rb * SC], BF16,
                                     isOutput=False)
    rs_d = nc.declare_dram_parameter("rs", [4, KD, TOTBs * HB], BF16,
                                     isOutput=False)
    ws_d = nc.declare_dram_parameter("ws", [4, KD, Rs * SC], BF16,
                                     isOutput=False)
    me_d = nc.declare_dram_parameter("mask_e", [128, Rb], F32, isOutput=False)
    mi_d = nc.declare_dram_parameter("mask_i", [128, Rb], F32, isOutput=False)
    mo_d = nc.declare_dram_parameter("mask_o", [128, Rs], F32, isOutput=False)
    xr_d = nc.declare_dram_parameter("xres", [128, Rb], F32, isOutput=False)
    yr_d = nc.declare_dram_parameter("yres", [128, Rs], F32, isOutput=False)
    out_d = nc.declare_dram_parameter("out", [128, 4], F32, isOutput=True)

    with ExitStack() as ctx:
        tc = ctx.enter_context(tile.TileContext(nc))
        singles = ctx.enter_context(tc.tile_pool(name="singles", bufs=1))
        d16p = ctx.enter_context(tc.tile_pool(name="d16p", bufs=4))
        l1p = ctx.enter_context(tc.tile_pool(name="l1p", bufs=2))
        l2p = ctx.enter_context(tc.tile_pool(name="l2p", bufs=2))
        l3p = ctx.enter_context(tc.tile_pool(name="l3p", bufs=2))

        RB = singles.tile([128, TOTBb, HB], BF16)
        WB = singles.tile([128, Rb, SC], BF16)
        RS = singles.tile([128, TOTBs, HB], BF16)
        WS = singles.tile([128, Rs, SC], BF16)
        RBf = RB.rearrange("p a b -> p (a b)")
        WBf = WB.rearrange("p a b -> p (a b)")
        RSf = RS.rearrange("p a b -> p (a b)")
        WSf = WS.rearrange("p a b -> p (a b)")
        # scalar queue must stay free for drains: bulk DMA on sync+gpsimd
        dq = [nc.sync, nc.gpsimd]
        for c in range(4):
            dq[c % 2].dma_start(out=WBf[32 * c:32 * c + KD, :], in_=wb_d[c])
        # rhs cells, chunked; chunk0 of all classes first for fast start.
        qi = [0]

        def rchunks(totb, RF, r_d, queues, early=0, first=8):
            chunks = [(0, min(first, totb))]
            lo = chunks[0][1]
            while lo < totb:
                hi = min(lo + 24, totb)
                chunks.append((lo, hi))
                lo = hi
            for k, (lo, hi) in enumerate(chunks):
                for c in range(4):
                    if k < early and c >= 2:
                        q = nc.scalar  # idle until first drain
                    else:
                        q = queues[qi[0] % len(queues)]
                        qi[0] += 1
                    q.dma_start(
                        out=RF[32 * c:32 * c + KD, lo * HB:hi * HB],
                        in_=r_d[c, :, lo * HB:hi * HB])

        rchunks(TOTBb, RBf, rb_d, [nc.sync, nc.gpsimd], early=2, first=4)
        # small-pass weights land after the big-pass stream is underway
        for c in range(4):
            dq[(c + 1) % 2].dma_start(
                out=WSf[32 * c:32 * c + KD, :], in_=ws_d[c])
        rchunks(TOTBs, RSf, rs_d, [nc.scalar, nc.sync, nc.gpsimd, nc.sync])
        # prewarm ACT spline tables (sqrt/tanh) while DMAs land
        warm = singles.tile([128, 1], F32)
        nc.vector.memset(warm, 1.0)
        nc.scalar.sqrt(warm, warm)
        nc.scalar.activation(warm, warm, AF.Tanh, scale=1.0)

        me = singles.tile([128, Rb], F32)
        nc.scalar.dma_start(out=me, in_=me_d[:, :])
        mi = singles.tile([128, Rb], F32)
        nc.scalar.dma_start(out=mi, in_=mi_d[:, :])
        mo = singles.tile([128, Rs], F32)
        nc.scalar.dma_start(out=mo, in_=mo_d[:, :])
        xr = singles.tile([128, Rb], F32)
        nc.scalar.dma_start(out=xr, in_=xr_d[:, :])
        yr = singles.tile([128, Rs], F32)
        nc.scalar.dma_start(out=yr, in_=yr_d[:, :])

        MPB = singles.tile([128, TOTBb + 8], F32)  # per-half partial minima
        MPS = singles.tile([128, TOTBs + 8], F32)
        MHB = singles.tile([128, Rb], F32)  # per-rank minima
        MHS = singles.tile([128, Rs], F32)

        act_t = [0.0]
        dve_t = [0.0]

        def consume(ps, H, NHW, MP, h0):
            # dst: MP[:, h0:h0+NHW] per-half minima (+1 pad col if odd)
            nout = 2 * H
            dst = MP[:, h0:h0 + nout]
            aA = (512 * H + 352) / 1.2
            dA = (232 + 288 * H) / 0.96
            dC = (120 + 512 * H) / 0.96
            costA = max(act_t[0] + aA, dve_t[0] + dA)
            costC = max(act_t[0], dve_t[0] + dC)
            if costC < costA:
                psv = ps[:, 0:H, :].rearrange("p h (t x) -> p (h t) x", t=2)
                nc.vector.tensor_reduce(dst, psv, axis=AX, op=MIN)
                dve_t[0] += dC
                return
            d16 = d16p.tile([128, 4, BK], F16)
            nc.scalar.copy(d16[:, 0:H, :], ps[:, 0:H, :])
            l1 = l1p.tile([128, 4, 2, BK // 4], F16)
            d16v = d16.rearrange("p h (t x) -> p h t x", t=2)
            nc.vector.tensor_tensor(
                l1[:, 0:H, :, :], d16v[:, 0:H, :, 0:BK // 4],
                d16v[:, 0:H, :, BK // 4:BK // 2], MIN)
            l2 = l2p.tile([128, 4, 2, BK // 8], F16)
            nc.vector.tensor_tensor(
                l2[:, 0:H, :, :], l1[:, 0:H, :, 0:BK // 8],
                l1[:, 0:H, :, BK // 8:BK // 4], MIN)
            l3 = l3p.tile([128, 4, 2, BK // 16], F16)
            nc.vector.tensor_tensor(
                l3[:, 0:H, :, :], l2[:, 0:H, :, 0:BK // 16],
                l2[:, 0:H, :, BK // 16:BK // 8], MIN)
            l3v = l3.rearrange("p h t x -> p (h t) x")
            nc.vector.tensor_reduce(dst, l3v[:, 0:nout, :], axis=AX, op=MIN)
            act_t[0] += aA
            dve_t[0] += dA

        def run_pass(TOTH, half_rank, RT, WT, MP, MH, ranges):
            # halves of 256 cols; wave = up to 8 halves = 4 PSUM banks
            nw = (TOTH + 3) // 4
            with tc.tile_pool(name="ps", bufs=4, space="PSUM") as psp:
                for w in range(nw):
                    h0 = 4 * w
                    NHW = min(4, TOTH - h0)
                    H = (NHW + 1) // 2
                    ps = psp.tile([128, 2, BK], F32)
                    for c in range(4):
                        for j in range(NHW):
                            r = half_rank[h0 + j]
                            nc.tensor.matmul(
                                ps[32 * c:32 * c + 32, j // 2,
                                   (j % 2) * HB:(j % 2) * HB + HB],
                                WT[32 * c:32 * c + KD, r * SC:(r + 1) * SC],
                                RT[32 * c:32 * c + KD,
                                   (h0 + j) * HB:(h0 + j + 1) * HB],
                                start=True, stop=True,
                                tile_position=(32 * c, 32 * c),
                            )
                        if NHW % 2 == 1:
                            # fill odd tail with a repeat (keeps reduce valid)
                            r = half_rank[h0 + NHW - 1]
                            nc.tensor.matmul(
                                ps[32 * c:32 * c + 32, NHW // 2, HB:2 * HB],
                                WT[32 * c:32 * c + KD, r * SC:(r + 1) * SC],
                                RT[32 * c:32 * c + KD,
                                   (h0 + NHW - 1) * HB:(h0 + NHW) * HB],
                                start=True, stop=True,
                                tile_position=(32 * c, 32 * c),
                            )
                    consume(ps, H, NHW, MP, h0)
                    # emit rank reduces whose halves completed in this wave
                    for r, (lo, hi) in enumerate(ranges):
                        if h0 < hi <= h0 + NHW:
                            nc.vector.tensor_reduce(
                                MH[:, r:r + 1], MP[:, lo:hi], axis=AX, op=MIN)

        outsb = singles.tile([128, 4], F32)
        jh = singles.tile([128, Rb], F32)
        jh2 = singles.tile([128, Rb], F32)
        jo = singles.tile([128, Rs], F32)
        run_pass(TOTBb, plan["bank_rank_big"], RBf, WBf, MPB, MHB,
                 plan["ranges_big"])
        run_pass(TOTBs, plan["bank_rank_small"], RSf, WSf, MPS, MHS,
                 plan["ranges_small"])
        # end phase, batched by ACT table set
        nc.vector.tensor_tensor(MHB, MHB, xr, ADD)
        nc.vector.tensor_tensor(MHS, MHS, yr, ADD)
        for MH in (MHB, MHS):
            nc.vector.tensor_scalar_max(MH, MH, 0.0)
            nc.vector.tensor_scalar_min(MH, MH, 1.0e4)
        nc.scalar.sqrt(MHB, MHB)
        nc.scalar.sqrt(MHS, MHS)
        nc.scalar.activation(MHB, MHB, AF.Tanh, scale=1.0 / (0.025 * float(CS)))
        nc.scalar.activation(MHS, MHS, AF.Tanh, scale=1.0 / (0.025 * float(CS)))
        nc.vector.tensor_tensor(jh, MHB, me, MULT)
        nc.vector.tensor_reduce(outsb[:, 0:1], jh, axis=AX, op=ADD)
        nc.vector.tensor_tensor(jh2, MHB, mi, MULT)
        nc.vector.tensor_reduce(outsb[:, 1:2], jh2, axis=AX, op=ADD)
        nc.vector.tensor_tensor(jo, MHS, mo, MULT)
        nc.vector.tensor_reduce(outsb[:, 2:3], jo, axis=AX, op=ADD)
        nc.vector.memset(outsb[:, 3:4], 0.0)
        nc.sync.dma_start(out=out_d[:, :], in_=outsb)
    nc.compile()
    return nc


def _get_nc(plan):
    key = plan["sig"]
    if key not in _nc_cache:
        _nc_cache[key] = _build(plan)
    return _nc_cache[key]


# ---------------------------------------------------------------- kernel
def kernel(hand_verts, obj_verts, obj_split_sizes, exterior_hand, exterior_obj):
    import ml_dtypes

    hv = np.ascontiguousarray(hand_verts, dtype=np.float32) * CS
    ov = np.ascontiguousarray(obj_verts, dtype=np.float32) * CS
    splits = np.asarray(obj_split_sizes).astype(np.int64).reshape(B)
    eh = np.asarray(exterior_hand).astype(bool).reshape(B, NH)
    eo = np.asarray(exterior_obj).astype(bool).reshape(B, NO)
    valid = np.arange(NO)[None, :] < splits[:, None]
    interior = (~eo) & valid

    xx = (hv * hv).sum(-1).astype(np.float32)
    yy = (ov * ov).sum(-1).astype(np.float32)

    def split3(x):
        x0 = x.astype(ml_dtypes.bfloat16).astype(np.float32)
        r = x - x0
        x1 = r.astype(ml_dtypes.bfloat16).astype(np.float32)
        return x0, x1, r - x1

    o0, o1, o2 = split3(ov)
    h0, h1, h2 = split3(hv)
    y0, y1, y2 = split3(yy)
    x0, x1, x2 = split3(xx)
    xres = x1 + x2  # per-hand-vert residual, added post-min
    yres = y1 + y2  # per-obj-vert residual, added post-min
    A_SEQ = [o0, o0, o1, o1, o0, o2]
    B_SEQ = [h0, h1, h0, h1, h2, h0]

    # ---------- per-batch geometry plans ----------
    plans = []
    for b in range(B):
        v = int(splits[b])
        hvd = hv[b].astype(np.float64) / float(CS)
        ovd = ov[b, :v].astype(np.float64) / float(CS)
        hord = _kd_order(hvd, SC)
        oord = _kd_order(ovd, FB)
        hs = hvd[hord]
        os_ = ovd[oord]
        big_need = _needed_blocks(hs, os_, os_[::4])
        io_mask = interior[b, :v][oord]
        iidx = np.nonzero(io_mask)[0]
        ipts = os_[iidx]
        iord2 = _kd_order(ipts, SC)
        ipts = ipts[iord2]
        iglob = oord[iidx[iord2]]
        small_need = _needed_blocks(ipts, hs, hs[::2])
        nbk_big = [max(1, (len(n) + BPH - 1) // BPH) for n in big_need]
        nbk_small = [max(1, (len(n) + BPH - 1) // BPH) for n in small_need]
        plans.append({
            "v": v, "hord": hord, "oord": oord, "iglob": iglob,
            "big_need": big_need, "small_need": small_need,
            "nbk_big": nbk_big, "nbk_small": nbk_small,
            "load": sum(nbk_big) + sum(nbk_small),
        })

    # ---------- batch -> (core, slot) snake by load ----------
    order = np.argsort([-plans[b]["load"] for b in range(B)], kind="stable")
    slot_batches = np.empty((BPC, NCORES), np.int64)
    for s in range(BPC):
        seg = order[s * NCORES:(s + 1) * NCORES]
        if s % 2 == 1:
            seg = seg[::-1]
        slot_batches[s] = seg

    # ---------- unified rank-pure schedule ----------
    def unify(pass_key, nranks_per_slot):
        # Snake-deal sub-chunks (sorted desc by bank count) into the 4
        # classes so every class's rank-r count is as similar as possible.
        rank_maps = {}
        nbk = []
        for s in range(BPC):
            nr = nranks_per_slot[s]
            ranks = [1] * nr
            for core in range(NCORES):
                p = plans[slot_batches[s, core]]
                counts = p[pass_key]
                order_q = sorted(range(len(counts)), key=lambda q: -counts[q])
                percls = [[] for _ in range(4)]
                for r, q in enumerate(order_q):
                    c = r % 4 if (r // 4) % 2 == 0 else 3 - r % 4
                    percls[c].append(q)
                for c in range(4):
                    rank_maps[(core, s, c)] = percls[c]
                    for r, q in enumerate(percls[c]):
                        if r < nr:
                            ranks[r] = max(ranks[r], counts[q])
            nbk.append(ranks)
        bank_rank = []
        ranges = []
        slot_rank_base = []
        rglob = 0
        for s in range(BPC):
            slot_rank_base.append(rglob)
            for r, nb in enumerate(nbk[s]):
                lo = len(bank_rank)
                bank_rank.extend([rglob] * nb)
                ranges.append((lo, lo + nb))
                rglob += 1
        return (rank_maps, nbk, bank_rank, ranges, len(bank_rank), rglob,
                slot_rank_base)

    nranks_b = [max(len(range(c, NSC, 4)) for c in range(4))
                for s in range(BPC)]  # 7
    (rmap_b, nbk_b, bank_rank_b, ranges_b, TOTBb, Rb, srb_b) = \
        unify("nbk_big", nranks_b)

    nranks_s = []
    for s in range(BPC):
        mx = 1
        for core in range(NCORES):
            p = plans[slot_batches[s, core]]
            for c in range(4):
                mx = max(mx, len(range(c, len(p["nbk_small"]), 4)))
        nranks_s.append(mx)
    (rmap_s, nbk_s, bank_rank_s, ranges_s, TOTBs, Rs, srb_s) = \
        unify("nbk_small", nranks_s)

    plan = {
        "totb_big": TOTBb, "totb_small": TOTBs,
        "r_big": Rb, "r_small": Rs,
        "bank_rank_big": bank_rank_b, "bank_rank_small": bank_rank_s,
        "ranges_big": ranges_b, "ranges_small": ranges_s,
    }
    plan["sig"] = repr((TOTBb, TOTBs, Rb, Rs, bank_rank_b, bank_rank_s))

    # ---------- per-core packing ----------
    in_maps = []
    for core in range(NCORES):
        rb = np.zeros((4, KD, TOTBb * HB), np.float32)
        wb = np.zeros((4, KD, Rb * SC), np.float32)
        rs = np.zeros((4, KD, TOTBs * HB), np.float32)
        ws = np.zeros((4, KD, Rs * SC), np.float32)
        me = np.zeros((128, Rb), np.float32)
        mi = np.zeros((128, Rb), np.float32)
        mo = np.zeros((128, Rs), np.float32)
        xr = np.zeros((128, Rb), np.float32)
        yr = np.zeros((128, Rs), np.float32)
        for s in range(BPC):
            b = int(slot_batches[s, core])
            p = plans[b]
            v = p["v"]
            hord, oord = p["hord"], p["oord"]
            nop = ((v + FB - 1) // FB) * FB
            # big pass: rhs = obj cols, lhsT = hand lanes
            ocols = np.zeros((KD, nop), np.float32)
            for j in range(6):
                for d in range(3):
                    ocols[3 * j + d, :v] = A_SEQ[j][b, oord, d]
            ocols[18, :v] = y0[b, oord]
            ocols[18, v:] = BIG
            ocols[19, :v] = y1[b, oord]
            ocols[20, :v] = y2[b, oord]
            ocols[21, :v] = 1.0
            hlanes = np.zeros((KD, NHP), np.float32)
            for j in range(6):
                for d in range(3):
                    hlanes[3 * j + d, :NH] = -2.0 * B_SEQ[j][b, hord, d]
            hlanes[18:21, :NH] = 1.0
            hlanes[21, :NH] = x0[b, hord]
            ehb = eh[b][hord]
            xresb = xres[b, hord]

            # small pass: rhs = hand cols, lhsT = interior obj lanes
            hcols = np.zeros((KD, NHP), np.float32)
            for j in range(6):
                for d in range(3):
                    hcols[3 * j + d, :NH] = -2.0 * B_SEQ[j][b, hord, d]
            hcols[18, :NH] = x0[b, hord]
            hcols[18, NH:] = PAD
            hcols[19, :NH] = x1[b, hord]
            hcols[20, :NH] = x2[b, hord]
            hcols[21, :NH] = 1.0
            iglob = p["iglob"]
            ni = len(iglob)
            olanes = np.zeros((KD, max(ni, 1)), np.float32)
            for j in range(6):
                for d in range(3):
                    olanes[3 * j + d, :ni] = A_SEQ[j][b, iglob, d]
            olanes[18:21, :ni] = 1.0
            olanes[21, :ni] = y0[b, iglob]
            yresb = yres[b, iglob]

            def pack(rank_maps, nbk_slot, rbase, ranges_of, need_key, cols,
                     lanes, nlanes_of, rarr, warr, mask_sinks):
                for c in range(4):
                    subs = rank_maps[(core, s, c)]
                    for r in range(len(nbk_slot)):
                        rg = rbase + r
                        nbu = nbk_slot[r]
                        lo = ranges_of[rg][0]
                        if r < len(subs):
                            q = subs[r]
                            nl = nlanes_of(q)
                            warr[c, :, rg * SC:rg * SC + nl] = \
                                lanes[:, q * SC:q * SC + nl]
                            blocks = p[need_key][q]
                            nblk = len(blocks)
                            tot = nbu * BPH
                            gath = [blocks[j % nblk] for j in range(tot)]
                            gcols = np.concatenate(
                                [np.arange(g * FB, (g + 1) * FB)
                                 for g in gath])
                            rarr[c, :, lo * HB:lo * HB + tot * FB] = \
                                cols[:, gcols]
                            for msk, vals in mask_sinks(q, nl):
                                msk[32 * c:32 * c + nl, rg] = vals
                        else:
                            rarr[c, :, lo * HB:(lo + nbu) * HB] = \
                                np.tile(cols[:, 0:HB], (1, nbu))

            pack(rmap_b, nbk_b[s], srb_b[s], ranges_b, "big_need", ocols,
                 hlanes, lambda q: min(SC, NH - q * SC), rb, wb,
                 lambda q, nl: [(me, ehb[q * SC:q * SC + nl]),
                                (mi, ~ehb[q * SC:q * SC + nl]),
                                (xr, xresb[q * SC:q * SC + nl])])
            pack(rmap_s, nbk_s[s], srb_s[s], ranges_s, "small_need", hcols,
                 olanes, lambda q: min(SC, ni - q * SC), rs, ws,
                 lambda q, nl: [(mo, 1.0),
                                (yr, yresb[q * SC:q * SC + nl])])

        in_maps.append({
            "rb": rb.astype(ml_dtypes.bfloat16),
            "wb": wb.astype(ml_dtypes.bfloat16),
            "rs": rs.astype(ml_dtypes.bfloat16),
            "ws": ws.astype(ml_dtypes.bfloat16),
            "mask_e": me, "mask_i": mi, "mask_o": mo,
            "xres": xr, "yres": yr,
        })

    nc = _get_nc(plan)
    res = run_bass_kernel_spmd(nc, in_maps, list(range(NCORES))).results

    nums = np.zeros(3, np.float64)
    for rr in res:
        nums += rr["out"][:, 0:3].astype(np.float64).sum(axis=0)
    dens = np.array([eh.sum(), (~eh).sum(), interior.sum()], dtype=np.float64)
    out = np.where(dens > 0, 0.025 * nums / np.maximum(dens, 1.0), 0.0)
    return out.astype(np.float32)
